# revision 21
# baseline (speedup 1.0000x reference)
"""Trainium2 Bass kernel for nn_BagInput (segment_reduce + linear/relu + BatchNorm).

Computation (matches the reference):
    h   = relu(x @ W.T + b)                      [N_items, 128]
    agg = segment_mean(h, seg_ids, NB)           [NB, 128]   (empty bags -> 0)
    out = (agg - mean) * rsqrt(var + eps) * gamma + beta   (batch stats over bags)

Strategy (8 NeuronCores, data-parallel over items, bag-aligned shards):
  - Host: shard items at bag boundaries; per core, pack items into groups of
    T0=16 128-item tiles, padding each group so that group boundaries fall on
    bag boundaries (~0.5% padding). One group == one "window" of <=128 bags.
  - Host: pre-transpose x to feature-major (xT) so the per-tile [feat, item]
    blocks DMA directly as matmul stationary operands. x is cast to bf16
    (mode 3, default): the rel-err budget (2e-2) dwarfs bf16 rounding
    (~0.5%), and it halves HBM traffic vs hi/lo-split f32 emulation.
  - Device per 128-item tile:
      h_psum = xT0_t.T @ WT0 + xT1_t.T @ WT1      (PE, K=256 in 2 chunks)
      h_sbuf = relu(h_psum) -> bf16               (DVE / ACT alternating)
      S      = (iota_row == slot_ids_t)           (0/1 selector, DVE, bf16)
      wps_w += S.T @ h_sbuf                       (PE; window accumulate in PSUM)
  - Window drain: agg = wps * (1/cnt)  (per-partition scalar); accumulate BN
    partial stats via mask.T @ [agg, agg^2] matmuls into PSUM.
  - AllReduce (8 cores) of the [1,256] stats; compute A = gamma*rsqrt(var+eps),
    B = beta - mean*A; broadcast via ones-matmul; out = agg*A + B; DMA out.
  - Host: gather per-(window,slot) rows back to global bag order.
"""

import numpy as np

N_CORES = 8
TILE = 128
FEAT = 256
BAG = 128
EPS = 1e-5

_NC_CACHE = {}
LAST_RESULTS = None  # BassKernelResults of the most recent run (for profiling)


# ----------------------------------------------------------------------------
# Host-side planning
# ----------------------------------------------------------------------------

def _plan_cores(seg_ids, n_bags, t0):
    """Split items/bags across cores at bag boundaries; pack groups of t0
    tiles per core such that each group covers whole bags (<=128 bags)."""
    gi = t0 * TILE
    n = seg_ids.shape[0]
    cuts = [0]
    bag_cuts = [0]
    for c in range(1, N_CORES):
        tgt = (n * c) // N_CORES
        bb = int(seg_ids[tgt])
        cut = int(np.searchsorted(seg_ids, bb, side="left"))
        cuts.append(cut)
        bag_cuts.append(bb)
    cuts.append(n)
    bag_cuts.append(n_bags)
    for c in range(N_CORES):
        if cuts[c + 1] <= cuts[c]:
            raise ValueError("degenerate core split")

    cores = []
    for c in range(N_CORES):
        i0, i1 = cuts[c], cuts[c + 1]
        b0, b1 = bag_cuts[c], bag_cuts[c + 1]
        seg = seg_ids[i0:i1]
        nloc = i1 - i0
        groups = []
        p = 0
        fb = b0
        while p < nloc:
            if p + gi >= nloc:
                e = nloc
                lbx = b1
            else:
                e = int(np.searchsorted(seg, seg[p + gi], side="left"))
                if e <= p:
                    raise ValueError("single bag larger than group size")
                lbx = int(seg[e - 1]) + 1
            if lbx - fb > TILE:
                raise ValueError(f"window spans {lbx - fb} bags > {TILE}")
            groups.append((p, e, fb, lbx))
            fb = lbx
            p = e
        cores.append(dict(i0=i0, i1=i1, b0=b0, b1=b1, groups=groups))
    return cores


def _host_prep(x, W, b, gamma, beta, seg_ids, bags_len, bf_mode):
    n_bags = bags_len.shape[0]
    plan = None
    for t0 in (16, 8, 4):
        try:
            plan = _plan_cores(seg_ids, n_bags, t0)
            break
        except ValueError:
            continue
    if plan is None:
        raise ValueError("could not plan groups")
    gi = t0 * TILE

    ng = max(len(c["groups"]) for c in plan)
    nt = ng * t0
    npad = ng * gi

    cnt = np.maximum(bags_len, 1).astype(np.float32)
    recip_all = 1.0 / cnt

    import ml_dtypes
    bf = ml_dtypes.bfloat16

    in_maps = []
    for c in range(N_CORES):
        info = plan[c]
        i0 = info["i0"]
        seg = seg_ids[i0:info["i1"]]
        groups = info["groups"]

        idx = np.full(npad, -1, dtype=np.int64)
        slots = np.full(npad, 255.0, dtype=np.float32)
        recip = np.ones((ng, TILE), dtype=np.float32)
        mask = np.zeros((ng, TILE), dtype=np.float32)
        for g, (p, e, fb, lbx) in enumerate(groups):
            m = e - p
            idx[g * gi: g * gi + m] = i0 + p + np.arange(m)
            slots[g * gi: g * gi + m] = (seg[p:e] - fb).astype(np.float32)
            ns = lbx - fb
            recip[g, :ns] = recip_all[fb:lbx]
            mask[g, :ns] = 1.0

        xp = np.zeros((npad, FEAT), dtype=np.float32)
        valid = idx >= 0
        xp[valid] = x[idx[valid]]
        xT = np.ascontiguousarray(xp.T)
        del xp

        common = {
            "recip": np.ascontiguousarray(recip.T),
            "mask": np.ascontiguousarray(mask.T),
            "gamma_row": np.ascontiguousarray(gamma.reshape(1, BAG)),
            "beta_row": np.ascontiguousarray(beta.reshape(1, BAG)),
            "bias_bc": np.ascontiguousarray(
                np.tile(b.reshape(1, BAG), (TILE, 1))),
        }
        if bf_mode >= 3:
            # pure bf16: single-precision x and W, halve HBM traffic
            in_maps.append({
                "xTh": np.ascontiguousarray(xT.astype(bf)),
                "WTh": np.ascontiguousarray(W.T.astype(bf)),
                "slots": np.ascontiguousarray(slots.reshape(nt, TILE).T.astype(bf)),
                "iota": np.ascontiguousarray(
                    np.tile(np.arange(TILE, dtype=np.float32), (TILE, 1)).astype(bf)),
                **common,
            })
        elif bf_mode >= 1:
            xTh = xT.astype(bf)
            xTl = (xT - xTh.astype(np.float32)).astype(bf)
            WTf = np.ascontiguousarray(W.T)
            WTh = WTf.astype(bf)
            WTl = (WTf - WTh.astype(np.float32)).astype(bf)
            in_maps.append({
                "xTh": np.ascontiguousarray(xTh),
                "xTl": np.ascontiguousarray(xTl),
                "WTh": np.ascontiguousarray(WTh),
                "WTl": np.ascontiguousarray(WTl),
                "slots": np.ascontiguousarray(slots.reshape(nt, TILE).T.astype(bf)),
                "iota": np.ascontiguousarray(
                    np.tile(np.arange(TILE, dtype=np.float32), (TILE, 1)).astype(bf)),
                **common,
            })
        else:
            in_maps.append({
                "xT": xT,
                "slots": np.ascontiguousarray(slots.reshape(nt, TILE).T),
                "WT": np.ascontiguousarray(W.T),
                "iota": np.ascontiguousarray(
                    np.tile(np.arange(TILE, dtype=np.float32), (TILE, 1))),
                **common,
            })
    return plan, t0, ng, in_maps, n_bags


# ----------------------------------------------------------------------------
# Mode 4: transposed seg-matmul, 64-slot windows, recip folded into x
# ----------------------------------------------------------------------------

T0_T = 8              # tiles per window (1024 items)
SLOT_T = 64           # one-hot width (<=64 bags per window)


def _plan_cores_t(seg_ids, n_bags):
    """Core split at bag boundaries; per core, windows of <=T0_T*128 items
    AND <=SLOT_T bags, each window covering whole bags."""
    gi = T0_T * TILE
    n = seg_ids.shape[0]
    cuts = [0]
    bag_cuts = [0]
    for c in range(1, N_CORES):
        tgt = (n * c) // N_CORES
        bb = int(seg_ids[tgt])
        cut = int(np.searchsorted(seg_ids, bb, side="left"))
        cuts.append(cut)
        bag_cuts.append(bb)
    cuts.append(n)
    bag_cuts.append(n_bags)
    for c in range(N_CORES):
        if cuts[c + 1] <= cuts[c]:
            raise ValueError("degenerate core split")

    cores = []
    for c in range(N_CORES):
        i0, i1 = cuts[c], cuts[c + 1]
        b0, b1 = bag_cuts[c], bag_cuts[c + 1]
        seg = seg_ids[i0:i1]
        nloc = i1 - i0
        groups = []
        p = 0
        fb = b0
        while p < nloc:
            if p + gi >= nloc:
                e = nloc
                lbx = b1
            else:
                e = int(np.searchsorted(seg, seg[p + gi], side="left"))
                if e <= p:
                    raise ValueError("single bag larger than window")
                lbx = int(seg[e - 1]) + 1
            if lbx - fb > SLOT_T:
                lbx = fb + SLOT_T
                e = int(np.searchsorted(seg, lbx, side="left"))
                if e <= p:
                    raise ValueError("bag-capped window is empty")
            groups.append((p, e, fb, lbx))
            fb = lbx
            p = e
        cores.append(dict(i0=i0, i1=i1, b0=b0, b1=b1, groups=groups))
    return cores


def _host_prep_t(x, W, gamma, beta, seg_ids, bags_len):
    n_bags = bags_len.shape[0]
    plan = _plan_cores_t(seg_ids, n_bags)
    gi = T0_T * TILE

    ng = max(len(c["groups"]) for c in plan)
    if ng % 2:
        ng += 1
    nt = ng * T0_T
    npad = ng * gi

    cnt = np.maximum(bags_len, 1).astype(np.float32)
    recip_all = (1.0 / cnt).astype(np.float32)
    xs = x * recip_all[np.asarray(seg_ids)][:, None]

    import ml_dtypes
    bf = ml_dtypes.bfloat16

    WT_bf = np.ascontiguousarray(W.T.astype(bf))
    iota64 = np.ascontiguousarray(
        np.tile(np.arange(SLOT_T, dtype=np.float32), (TILE, 1)).astype(bf))

    in_maps = []
    for c in range(N_CORES):
        info = plan[c]
        i0 = info["i0"]
        seg = seg_ids[i0:info["i1"]]
        groups = info["groups"]

        idx = np.full(npad, -1, dtype=np.int64)
        slots = np.full(npad, 255.0, dtype=np.float32)
        for g, (p, e, fb, lbx) in enumerate(groups):
            m = e - p
            idx[g * gi: g * gi + m] = i0 + p + np.arange(m)
            slots[g * gi: g * gi + m] = (seg[p:e] - fb).astype(np.float32)

        xp = np.zeros((npad, FEAT), dtype=np.float32)
        valid = idx >= 0
        xp[valid] = xs[idx[valid]]
        xT = np.ascontiguousarray(xp.T.astype(bf))
        del xp

        in_maps.append({
            "xT": xT,
            "WT": WT_bf,
            "slots": np.ascontiguousarray(slots.reshape(nt, TILE).T.astype(bf)),
            "iota64": iota64,
            "gamma_col": np.ascontiguousarray(gamma.reshape(BAG, 1)),
            "beta_col": np.ascontiguousarray(beta.reshape(BAG, 1)),
        })
    return plan, ng, in_maps, n_bags


def _build_nc_t(ng, n_bags):
    import os
    use_ttr = os.environ.get("KERNEL_TTR", "1") == "1"
    use_cc = os.environ.get("KERNEL_NOCC", "0") != "1"
    import concourse.bacc as bacc
    import concourse.tile as tile
    import concourse.mybir as mybir
    import concourse.bass as bass_mod

    fp32 = mybir.dt.float32
    bf16 = mybir.dt.bfloat16
    AOT = mybir.AluOpType
    AFT = mybir.ActivationFunctionType

    gi = T0_T * TILE
    nt = ng * T0_T
    npad = ng * gi
    np2 = ng // 2
    SW = T0_T * SLOT_T          # selector cols per window (512)

    nc = bacc.Bacc("TRN2", target_bir_lowering=False, debug=False,
                   enable_asserts=False, num_devices=N_CORES)
    xT = nc.dram_tensor("xT", [FEAT, npad], bf16, kind="ExternalInput")
    WT_in = nc.dram_tensor("WT", [FEAT, BAG], bf16, kind="ExternalInput")
    slots = nc.dram_tensor("slots", [TILE, nt], bf16, kind="ExternalInput")
    iota_in = nc.dram_tensor("iota64", [TILE, SLOT_T], bf16, kind="ExternalInput")
    gcol_in = nc.dram_tensor("gamma_col", [BAG, 1], fp32, kind="ExternalInput")
    bcol_in = nc.dram_tensor("beta_col", [BAG, 1], fp32, kind="ExternalInput")
    outT = nc.dram_tensor("outT", [TILE, np2 * TILE], bf16, kind="ExternalOutput")

    with tile.TileContext(nc) as tc:
        with tc.tile_pool(name="const", bufs=1) as constp, \
             tc.tile_pool(name="xa", bufs=5) as xa_p, \
             tc.tile_pool(name="xb", bufs=5) as xb_p, \
             tc.tile_pool(name="hsb", bufs=4) as hsb_p, \
             tc.tile_pool(name="Sp", bufs=4) as s_p, \
             tc.tile_pool(name="agg", bufs=1) as agg_p, \
             tc.tile_pool(name="sq", bufs=2) as sq_p, \
             tc.tile_pool(name="stat", bufs=4) as stat_p, \
             tc.tile_pool(name="outp", bufs=2) as out_p, \
             tc.tile_pool(name="small", bufs=1) as small_p, \
             tc.tile_pool(name="hps", bufs=3, space="PSUM") as hps_p, \
             tc.tile_pool(name="wpsp", bufs=2, space="PSUM") as wps_p, \
             tc.tile_pool(name="dram", bufs=1, space="DRAM") as dram_p:

            wt0 = constp.tile([128, BAG], bf16)
            nc.sync.dma_start(wt0[:], WT_in[0:128, :])
            wt1 = constp.tile([128, BAG], bf16)
            nc.sync.dma_start(wt1[:], WT_in[128:256, :])
            iota_sb = constp.tile([TILE, SLOT_T], bf16)
            nc.sync.dma_start(iota_sb[:], iota_in[:, :])
            slots_sb = constp.tile([TILE, nt], bf16)
            nc.sync.dma_start(slots_sb[:], slots[:, :])
            gcol = constp.tile([BAG, 1], fp32)
            nc.sync.dma_start(gcol[:], gcol_in[:, :])
            bcol = constp.tile([BAG, 1], fp32)
            nc.sync.dma_start(bcol[:], bcol_in[:, :])
            zcol = constp.tile([TILE, 1], fp32)
            nc.vector.memset(zcol[:], 0.0)

            agg_big = agg_p.tile([TILE, np2 * BAG], fp32)
            if not use_ttr:
                sa_tab = constp.tile([TILE, np2], fp32)
                sb_tab = constp.tile([TILE, np2], fp32)

            wps_tiles = {}
            prev = None
            sa_prev = 0.0
            sb_prev = 0.0
            for q in range(ng + 1):
                if q < ng:
                    w = q
                    xa = xa_p.tile([128, gi], bf16, tag="xa")
                    nc.sync.dma_start(xa[:], xT[0:128, w * gi:(w + 1) * gi])
                    xb = xb_p.tile([128, gi], bf16, tag="xb")
                    nc.sync.dma_start(xb[:], xT[128:256, w * gi:(w + 1) * gi])
                    if w % 2 == 0:
                        # full-bank tile so the pair's accumulation groups and
                        # the drain never share a PSUM bank with another pair
                        wt_full = wps_p.tile([TILE, 512], fp32)
                        wps_tiles[w // 2] = wt_full
                    HW2 = T0_T * BAG // 2            # 512 = one PSUM bank
                    hpsa = hps_p.tile([TILE, HW2], fp32, tag="hpsa")
                    hpsb = hps_p.tile([TILE, HW2], fp32, tag="hpsb")
                    for j in range(T0_T):
                        hp = hpsa if j < 4 else hpsb
                        o = ((j % 4) * BAG, (j % 4 + 1) * BAG)
                        nc.tensor.matmul(hp[:, o[0]:o[1]],
                                         xa[:, j * 128:(j + 1) * 128], wt0[:],
                                         start=True, stop=False)
                        nc.tensor.matmul(hp[:, o[0]:o[1]],
                                         xb[:, j * 128:(j + 1) * 128], wt1[:],
                                         start=False, stop=True)
                    hsb = hsb_p.tile([TILE, T0_T * BAG], bf16)
                    nc.scalar.activation(hsb[:, 0:HW2], hpsa[:], AFT.Relu)
                    nc.scalar.activation(hsb[:, HW2:2 * HW2], hpsb[:], AFT.Relu)
                    # selector: S[p, a*64 + f] = (iota64[f] == slots[p, 8w+a])
                    S = s_p.tile([TILE, SW], bf16)
                    scol = slots_sb[:, w * T0_T:(w + 1) * T0_T]
                    srep = bass_mod.AP(tensor=scol.tensor, offset=scol.offset,
                                       ap=[scol.ap[0], scol.ap[1], [0, SLOT_T]])
                    ibase = iota_sb[:]
                    irep = bass_mod.AP(tensor=ibase.tensor, offset=ibase.offset,
                                       ap=[ibase.ap[0], [0, T0_T], ibase.ap[1]])
                    nc.vector.tensor_tensor(
                        S[:].rearrange("p (a b) -> p a b", a=T0_T),
                        irep, srep, AOT.is_equal)
                    cur = (w, hsb, S)
                else:
                    cur = None
                if prev is not None:
                    pw, phsb, pS = prev
                    # windows of a pair land at 512B-aligned cols 0 and 128
                    h0 = (pw % 2) * BAG
                    k = pw // 2
                    wt_ps = wps_tiles[k]
                    for j in range(T0_T):
                        nc.tensor.matmul(wt_ps[:, h0:h0 + SLOT_T],
                                         phsb[:, j * BAG:(j + 1) * BAG],
                                         pS[:, j * SLOT_T:(j + 1) * SLOT_T],
                                         start=(j == 0), stop=(j == T0_T - 1))
                    if pw % 2 == 1:
                        aggsl = agg_big[:, k * BAG:(k + 1) * BAG]
                        zrep = bass_mod.AP(tensor=zcol.tensor, offset=zcol.offset,
                                           ap=[zcol.ap[0], [0, 2], [0, SLOT_T]])
                        wsrc = wt_ps[:]
                        wstr = bass_mod.AP(tensor=wsrc.tensor, offset=wsrc.offset,
                                           ap=[wsrc.ap[0], [BAG, 2], [1, SLOT_T]])
                        if use_ttr:
                            sa_new = stat_p.tile([TILE, 1], fp32, tag="sa")
                            nc.vector.tensor_tensor_reduce(
                                aggsl.rearrange("p (a b) -> p a b", a=2),
                                wstr, zrep, 1.0, sa_prev,
                                AOT.add, AOT.add, sa_new[:])
                            sq = sq_p.tile([TILE, BAG], fp32)
                            sb_new = stat_p.tile([TILE, 1], fp32, tag="sb")
                            nc.vector.tensor_tensor_reduce(
                                sq[:], aggsl, aggsl, 1.0, sb_prev,
                                AOT.mult, AOT.add, sb_new[:])
                            sa_prev = sa_new[:]
                            sb_prev = sb_new[:]
                        else:
                            nc.vector.scalar_tensor_tensor(
                                aggsl.rearrange("p (a b) -> p a b", a=2),
                                wstr, 1.0, zrep, AOT.mult, AOT.add,
                                accum_out=sa_tab[:, k:k + 1])
                            sq = sq_p.tile([TILE, BAG], fp32)
                            nc.vector.scalar_tensor_tensor(
                                sq[:], aggsl, 0.0, aggsl, AOT.add, AOT.mult,
                                accum_out=sb_tab[:, k:k + 1])
                        del wps_tiles[k]
                prev = cur

            # ---------------- stats all-reduce + params ----------------
            stats_sb = small_p.tile([TILE, 2], fp32)
            if use_ttr:
                nc.vector.tensor_copy(stats_sb[:, 0:1], sa_prev)
                nc.vector.tensor_copy(stats_sb[:, 1:2], sb_prev)
            else:
                nc.vector.tensor_reduce(stats_sb[:, 0:1], sa_tab[:],
                                        mybir.AxisListType.X, AOT.add)
                nc.vector.tensor_reduce(stats_sb[:, 1:2], sb_tab[:],
                                        mybir.AxisListType.X, AOT.add)
            gstats = small_p.tile([TILE, 2], fp32)
            if use_cc:
                cc_in = dram_p.tile([TILE, 2], fp32)
                cc_out = dram_p.tile([TILE, 2], fp32)
                nc.sync.dma_start(cc_in[:], stats_sb[:])
                nc.gpsimd.collective_compute(
                    "AllReduce", AOT.add,
                    replica_groups=[list(range(N_CORES))],
                    ins=[cc_in.opt()], outs=[cc_out.opt()])
                nc.sync.dma_start(gstats[:], cc_out[:])
            else:
                nc.vector.tensor_copy(gstats[:], stats_sb[:])

            inv_nb = 1.0 / float(n_bags)
            mean = small_p.tile([TILE, 1], fp32)
            nc.vector.tensor_scalar_mul(mean[:], gstats[:, 0:1], inv_nb)
            ex2 = small_p.tile([TILE, 1], fp32)
            nc.vector.tensor_scalar_mul(ex2[:], gstats[:, 1:2], inv_nb)
            m2 = small_p.tile([TILE, 1], fp32)
            nc.vector.tensor_tensor(m2[:], mean[:], mean[:], AOT.mult)
            vareps = small_p.tile([TILE, 1], fp32)
            nc.vector.tensor_tensor(vareps[:], ex2[:], m2[:], AOT.subtract)
            nc.vector.tensor_scalar_add(vareps[:], vareps[:], EPS)
            rec = small_p.tile([TILE, 1], fp32)
            nc.vector.reciprocal(rec[:], vareps[:])
            inv = small_p.tile([TILE, 1], fp32)
            nc.scalar.sqrt(inv[:], rec[:])
            acol = small_p.tile([TILE, 1], fp32)
            nc.vector.tensor_tensor(acol[:], inv[:], gcol[:], AOT.mult)
            mA = small_p.tile([TILE, 1], fp32)
            nc.vector.tensor_tensor(mA[:], mean[:], acol[:], AOT.mult)
            bcol2 = small_p.tile([TILE, 1], fp32)
            nc.vector.tensor_tensor(bcol2[:], bcol[:], mA[:], AOT.subtract)

            # ---------------- phase 2: normalize + store ----------------
            CH = 1024
            tot = np2 * BAG
            w2 = 0
            while w2 < tot:
                cw = min(CH, tot - w2)
                brep = bass_mod.AP(tensor=bcol2.tensor, offset=bcol2.offset,
                                   ap=[bcol2.ap[0], [0, cw]])
                ot = out_p.tile([TILE, CH], bf16)
                nc.vector.scalar_tensor_tensor(
                    ot[:, 0:cw], agg_big[:, w2:w2 + cw], acol[:], brep,
                    AOT.mult, AOT.add)
                nc.sync.dma_start(outT[:, w2:w2 + cw], ot[:, 0:cw])
                w2 += cw

    nc.compile()
    return nc


# ----------------------------------------------------------------------------
# Device kernel (pure bf16, mode 3)
# ----------------------------------------------------------------------------

def _build_nc_pure(ng, t0, n_bags, has_bias):
    import concourse.bacc as bacc
    import concourse.tile as tile
    import concourse.mybir as mybir
    import concourse.bass as bass_mod

    fp32 = mybir.dt.float32
    bf16 = mybir.dt.bfloat16
    AOT = mybir.AluOpType
    AFT = mybir.ActivationFunctionType

    gi = t0 * TILE
    nt = ng * t0

    npad = ng * gi

    nc = bacc.Bacc("TRN2", target_bir_lowering=False, debug=False,
                   enable_asserts=False, num_devices=N_CORES)
    xTh = nc.dram_tensor("xTh", [FEAT, npad], bf16, kind="ExternalInput")
    WTh_in = nc.dram_tensor("WTh", [FEAT, BAG], bf16, kind="ExternalInput")
    slots = nc.dram_tensor("slots", [TILE, nt], bf16, kind="ExternalInput")
    recip = nc.dram_tensor("recip", [TILE, ng], fp32, kind="ExternalInput")
    mask = nc.dram_tensor("mask", [TILE, ng], fp32, kind="ExternalInput")
    iota_in = nc.dram_tensor("iota", [TILE, TILE], bf16, kind="ExternalInput")
    grow_in = nc.dram_tensor("gamma_row", [1, BAG], fp32, kind="ExternalInput")
    brow_in = nc.dram_tensor("beta_row", [1, BAG], fp32, kind="ExternalInput")
    bb_in = nc.dram_tensor("bias_bc", [TILE, BAG], fp32, kind="ExternalInput")
    out = nc.dram_tensor("out", [ng * TILE, BAG], fp32, kind="ExternalOutput")

    with tile.TileContext(nc) as tc:
        with tc.tile_pool(name="const", bufs=1) as constp, \
             tc.tile_pool(name="xa", bufs=5) as xa_p, \
             tc.tile_pool(name="xb", bufs=5) as xb_p, \
             tc.tile_pool(name="hsb", bufs=6) as hsb_p, \
             tc.tile_pool(name="Sp", bufs=6) as s_p, \
             tc.tile_pool(name="agg", bufs=1) as agg_p, \
             tc.tile_pool(name="agg2", bufs=2) as agg2_p, \
             tc.tile_pool(name="outp", bufs=2) as out_p, \
             tc.tile_pool(name="small", bufs=1) as small_p, \
             tc.tile_pool(name="hps", bufs=3, space="PSUM") as hps_p, \
             tc.tile_pool(name="wpsp", bufs=2, space="PSUM") as wps_p, \
             tc.tile_pool(name="spsa", bufs=1, space="PSUM") as sps_a_p, \
             tc.tile_pool(name="spsb", bufs=1, space="PSUM") as sps_b_p, \
             tc.tile_pool(name="abps", bufs=1, space="PSUM") as ab_p, \
             tc.tile_pool(name="dram", bufs=1, space="DRAM") as dram_p:

            wt0 = constp.tile([128, BAG], bf16)
            nc.sync.dma_start(wt0[:], WTh_in[0:128, :])
            wt1 = constp.tile([128, BAG], bf16)
            nc.sync.dma_start(wt1[:], WTh_in[128:256, :])
            iota_sb = constp.tile([TILE, TILE], bf16)
            nc.sync.dma_start(iota_sb[:], iota_in[:, :])
            recip_sb = constp.tile([TILE, ng], fp32)
            nc.sync.dma_start(recip_sb[:], recip[:, :])
            mask_sb = constp.tile([TILE, ng], fp32)
            nc.sync.dma_start(mask_sb[:], mask[:, :])
            slots_sb = constp.tile([TILE, nt], bf16)
            nc.sync.dma_start(slots_sb[:], slots[:, :])
            ones_row = constp.tile([1, TILE], fp32)
            nc.vector.memset(ones_row[:], 1.0)
            grow = constp.tile([1, BAG], fp32)
            nc.sync.dma_start(grow[:], grow_in[:, :])
            brow = constp.tile([1, BAG], fp32)
            nc.sync.dma_start(brow[:], brow_in[:, :])
            if has_bias:
                bias_bc = constp.tile([TILE, BAG], fp32)
                nc.sync.dma_start(bias_bc[:], bb_in[:, :])

            stats_a = sps_a_p.tile([1, BAG], fp32)
            stats_b = sps_b_p.tile([1, BAG], fp32)
            agg_big = agg_p.tile([TILE, ng * BAG], fp32)

            # ---------------- phase 1: streamed quads (4 tiles each) -------
            QT = 4                  # tiles per quad
            assert t0 % QT == 0
            qpw = t0 // QT          # quads per window
            nq = nt // QT
            WID = QT * BAG          # 512

            wps_tiles = {}
            xa = xb = None
            prev = None
            for q in range(nq + 1):
                if q < nq:
                    w, jq = divmod(q, qpw)
                    if jq == 0:
                        xa = xa_p.tile([128, gi], bf16, tag="xa")
                        nc.sync.dma_start(xa[:], xTh[0:128, w * gi:(w + 1) * gi])
                        xb = xb_p.tile([128, gi], bf16, tag="xb")
                        nc.sync.dma_start(xb[:], xTh[128:256, w * gi:(w + 1) * gi])
                        wt_ps = wps_p.tile([TILE, BAG], fp32)
                        wps_tiles[w] = wt_ps
                    hps = hps_p.tile([TILE, WID], fp32)
                    for j in range(QT):
                        c0 = (jq * QT + j) * 128
                        o = (j * BAG, (j + 1) * BAG)
                        nc.tensor.matmul(hps[:, o[0]:o[1]],
                                         xa[:, c0:c0 + 128], wt0[:],
                                         start=True, stop=False)
                        nc.tensor.matmul(hps[:, o[0]:o[1]],
                                         xb[:, c0:c0 + 128], wt1[:],
                                         start=False, stop=True)
                    hsb = hsb_p.tile([TILE, WID], bf16)
                    if has_bias:
                        bias4 = bass_mod.AP(
                            tensor=bias_bc.tensor, offset=bias_bc.offset,
                            ap=[bias_bc.ap[0], [0, QT], bias_bc.ap[1]])
                        nc.vector.tensor_tensor(
                            hsb[:].rearrange("p (a b) -> p a b", a=QT),
                            hps[:].rearrange("p (a b) -> p a b", a=QT),
                            bias4, AOT.add)
                        nc.vector.tensor_scalar_max(hsb[:], hsb[:], 0.0)
                    else:
                        nc.scalar.activation(hsb[:], hps[:], AFT.Relu)
                    # wide selector: S[p, a*128 + f] = (iota[f] == slots[p, t0q+a])
                    S = s_p.tile([TILE, WID], bf16)
                    scol = slots_sb[:, q * QT:(q + 1) * QT]
                    srep = bass_mod.AP(tensor=scol.tensor, offset=scol.offset,
                                       ap=[scol.ap[0], scol.ap[1], [0, BAG]])
                    ibase = iota_sb[:]
                    irep = bass_mod.AP(tensor=ibase.tensor, offset=ibase.offset,
                                       ap=[ibase.ap[0], [0, QT], ibase.ap[1]])
                    nc.vector.tensor_tensor(
                        S[:].rearrange("p (a b) -> p a b", a=QT),
                        irep, srep, AOT.is_equal)
                    cur = (q, S, hsb, w, jq == 0, jq == qpw - 1)
                else:
                    cur = None
                if prev is not None:
                    pq, pS, phsb, pw, pfirst, plast = prev
                    for j in range(QT):
                        nc.tensor.matmul(wps_tiles[pw][:],
                                         pS[:, j * BAG:(j + 1) * BAG],
                                         phsb[:, j * BAG:(j + 1) * BAG],
                                         start=(pfirst and j == 0),
                                         stop=(plast and j == QT - 1))
                    if plast:
                        aggw = agg_big[:, pw * BAG:(pw + 1) * BAG]
                        nc.scalar.activation(aggw, wps_tiles[pw][:], AFT.Copy,
                                             scale=recip_sb[:, pw:pw + 1])
                        a2 = agg2_p.tile([TILE, BAG], fp32)
                        nc.vector.tensor_tensor(a2[:], aggw, aggw, AOT.mult)
                        nc.tensor.matmul(stats_a[:], mask_sb[:, pw:pw + 1], aggw,
                                         start=(pw == 0), stop=(pw == ng - 1))
                        nc.tensor.matmul(stats_b[:], mask_sb[:, pw:pw + 1], a2[:],
                                         start=(pw == 0), stop=(pw == ng - 1))
                        del wps_tiles[pw]
                prev = cur

            # ---------------- stats all-reduce + params ----------------
            stats_sb = small_p.tile([1, 2 * BAG], fp32)
            nc.vector.tensor_copy(stats_sb[0:1, 0:BAG], stats_a[:])
            nc.vector.tensor_copy(stats_sb[0:1, BAG:2 * BAG], stats_b[:])
            cc_in = dram_p.tile([1, 2 * BAG], fp32)
            cc_out = dram_p.tile([1, 2 * BAG], fp32)
            nc.sync.dma_start(cc_in[:], stats_sb[:])
            nc.gpsimd.collective_compute(
                "AllReduce", AOT.add,
                replica_groups=[list(range(N_CORES))],
                ins=[cc_in.opt()], outs=[cc_out.opt()])
            gstats = small_p.tile([1, 2 * BAG], fp32)
            nc.sync.dma_start(gstats[:], cc_out[:])

            inv_nb = 1.0 / float(n_bags)
            mean = small_p.tile([1, BAG], fp32)
            nc.vector.tensor_scalar_mul(mean[:], gstats[0:1, 0:BAG], inv_nb)
            ex2 = small_p.tile([1, BAG], fp32)
            nc.vector.tensor_scalar_mul(ex2[:], gstats[0:1, BAG:2 * BAG], inv_nb)
            m2 = small_p.tile([1, BAG], fp32)
            nc.vector.tensor_tensor(m2[:], mean[:], mean[:], AOT.mult)
            vareps = small_p.tile([1, BAG], fp32)
            nc.vector.tensor_tensor(vareps[:], ex2[:], m2[:], AOT.subtract)
            nc.vector.tensor_scalar_add(vareps[:], vareps[:], EPS)
            rec = small_p.tile([1, BAG], fp32)
            nc.vector.reciprocal(rec[:], vareps[:])
            inv = small_p.tile([1, BAG], fp32)
            nc.scalar.sqrt(inv[:], rec[:])
            ab_row = small_p.tile([1, 2 * BAG], fp32)
            nc.vector.tensor_tensor(ab_row[0:1, 0:BAG], inv[:], grow[:], AOT.mult)
            mA = small_p.tile([1, BAG], fp32)
            nc.vector.tensor_tensor(mA[:], mean[:], ab_row[0:1, 0:BAG], AOT.mult)
            nc.vector.tensor_tensor(ab_row[0:1, BAG:2 * BAG], brow[:], mA[:],
                                    AOT.subtract)
            ab_ps = ab_p.tile([TILE, 2 * BAG], fp32)
            nc.tensor.matmul(ab_ps[:], ones_row[:], ab_row[:], start=True, stop=True)
            ab_sb = constp.tile([TILE, 2 * BAG], fp32)
            nc.vector.tensor_copy(ab_sb[:], ab_ps[:])

            # ---------------- phase 2: normalize + store ----------------
            a_col = ab_sb[:, 0:BAG]
            b_col = ab_sb[:, BAG:2 * BAG]
            w2 = 0
            while w2 < ng:
                nw = min(4, ng - w2)
                wid2 = nw * BAG
                arep = bass_mod.AP(tensor=a_col.tensor, offset=a_col.offset,
                                   ap=[a_col.ap[0], [0, nw], a_col.ap[1]])
                brep = bass_mod.AP(tensor=b_col.tensor, offset=b_col.offset,
                                   ap=[b_col.ap[0], [0, nw], b_col.ap[1]])
                ot = out_p.tile([TILE, 4 * BAG], fp32)
                src = agg_big[:, w2 * BAG:(w2 + nw) * BAG]
                nc.vector.tensor_tensor(
                    ot[:, 0:wid2].rearrange("p (a b) -> p a b", a=nw),
                    src.rearrange("p (a b) -> p a b", a=nw), arep, AOT.mult)
                nc.vector.tensor_tensor(
                    ot[:, 0:wid2].rearrange("p (a b) -> p a b", a=nw),
                    ot[:, 0:wid2].rearrange("p (a b) -> p a b", a=nw),
                    brep, AOT.add)
                # out rows for nw windows are contiguous: [w2*128, (w2+nw)*128)
                nc.sync.dma_start(
                    out[w2 * TILE:(w2 + nw) * TILE, :].rearrange(
                        "(a p) b -> p a b", p=TILE),
                    ot[:, 0:wid2].rearrange("p (a b) -> p a b", a=nw))
                w2 += nw

    nc.compile()
    return nc


# ----------------------------------------------------------------------------
# Device kernel (legacy modes 0-2: f32 / bf16 hi-lo split)
# ----------------------------------------------------------------------------

def _build_nc(ng, t0, n_bags, has_bias, relu_dve_mod=2, sbuild_dve_mod=4,
              use_f32r=False, use_bf16h=False, use_bf16seg=False):
    import concourse.bacc as bacc
    import concourse.tile as tile
    import concourse.mybir as mybir

    fp32 = mybir.dt.float32
    mmdt = mybir.dt.float32r if use_f32r else fp32
    bf16 = mybir.dt.bfloat16
    AOT = mybir.AluOpType
    AFT = mybir.ActivationFunctionType

    gi = t0 * TILE
    nt = ng * t0
    npad = ng * gi

    nc = bacc.Bacc("TRN2", target_bir_lowering=False, debug=False,
                   enable_asserts=False, num_devices=N_CORES)
    if use_bf16h:
        xTh = nc.dram_tensor("xTh", [FEAT, npad], bf16, kind="ExternalInput")
        xTl = nc.dram_tensor("xTl", [FEAT, npad], bf16, kind="ExternalInput")
        WTh_in = nc.dram_tensor("WTh", [FEAT, BAG], bf16, kind="ExternalInput")
        WTl_in = nc.dram_tensor("WTl", [FEAT, BAG], bf16, kind="ExternalInput")
    else:
        xT = nc.dram_tensor("xT", [FEAT, npad], mmdt, kind="ExternalInput")
        WT = nc.dram_tensor("WT", [FEAT, BAG], mmdt, kind="ExternalInput")
    sldt = bf16 if use_bf16h else fp32
    slots = nc.dram_tensor("slots", [TILE, nt], sldt, kind="ExternalInput")
    recip = nc.dram_tensor("recip", [TILE, ng], fp32, kind="ExternalInput")
    mask = nc.dram_tensor("mask", [TILE, ng], fp32, kind="ExternalInput")
    iota_in = nc.dram_tensor("iota", [TILE, TILE], sldt, kind="ExternalInput")
    grow_in = nc.dram_tensor("gamma_row", [1, BAG], fp32, kind="ExternalInput")
    brow_in = nc.dram_tensor("beta_row", [1, BAG], fp32, kind="ExternalInput")
    bb_in = nc.dram_tensor("bias_bc", [TILE, BAG], fp32, kind="ExternalInput")
    out = nc.dram_tensor("out", [ng * TILE, BAG], fp32, kind="ExternalOutput")

    with tile.TileContext(nc) as tc:
        with tc.tile_pool(name="const", bufs=1) as constp, \
             tc.tile_pool(name="xa", bufs=4) as xa_p, \
             tc.tile_pool(name="xb", bufs=4) as xb_p, \
             tc.tile_pool(name="hsb", bufs=6) as hsb_p, \
             tc.tile_pool(name="Sp", bufs=6) as s_p, \
             tc.tile_pool(name="agg", bufs=1) as agg_p, \
             tc.tile_pool(name="agg2", bufs=2) as agg2_p, \
             tc.tile_pool(name="outp", bufs=2) as out_p, \
             tc.tile_pool(name="small", bufs=1) as small_p, \
             tc.tile_pool(name="hps", bufs=3, space="PSUM") as hps_p, \
             tc.tile_pool(name="wpsp", bufs=2, space="PSUM") as wps_p, \
             tc.tile_pool(name="spsa", bufs=1, space="PSUM") as sps_a_p, \
             tc.tile_pool(name="spsb", bufs=1, space="PSUM") as sps_b_p, \
             tc.tile_pool(name="abps", bufs=1, space="PSUM") as ab_p, \
             tc.tile_pool(name="dram", bufs=1, space="DRAM") as dram_p:

            if use_bf16h:
                wt0h = constp.tile([128, BAG], bf16)
                nc.sync.dma_start(wt0h[:], WTh_in[0:128, :])
                wt1h = constp.tile([128, BAG], bf16)
                nc.sync.dma_start(wt1h[:], WTh_in[128:256, :])
                wt0l = constp.tile([128, BAG], bf16)
                nc.sync.dma_start(wt0l[:], WTl_in[0:128, :])
                wt1l = constp.tile([128, BAG], bf16)
                nc.sync.dma_start(wt1l[:], WTl_in[128:256, :])
            else:
                wt0 = constp.tile([128, BAG], mmdt)
                nc.sync.dma_start(wt0[:], WT[0:128, :])
                wt1 = constp.tile([128, BAG], mmdt)
                nc.sync.dma_start(wt1[:], WT[128:256, :])
            iota_sb = constp.tile([TILE, TILE], sldt)
            nc.sync.dma_start(iota_sb[:], iota_in[:, :])
            recip_sb = constp.tile([TILE, ng], fp32)
            nc.sync.dma_start(recip_sb[:], recip[:, :])
            mask_sb = constp.tile([TILE, ng], fp32)
            nc.sync.dma_start(mask_sb[:], mask[:, :])
            slots_sb = constp.tile([TILE, nt], sldt)
            nc.sync.dma_start(slots_sb[:], slots[:, :])
            segdt = bf16 if use_bf16seg else mmdt
            zeros_f32 = constp.tile([TILE, TILE], fp32)
            nc.vector.memset(zeros_f32[:], 0.0)
            if use_f32r or use_bf16seg:
                zeros_S = constp.tile([TILE, TILE], segdt)
                nc.vector.tensor_copy(zeros_S[:], zeros_f32[:])
            else:
                zeros_S = zeros_f32
            ones_row = constp.tile([1, TILE], fp32)
            nc.vector.memset(ones_row[:], 1.0)
            grow = constp.tile([1, BAG], fp32)
            nc.sync.dma_start(grow[:], grow_in[:, :])
            brow = constp.tile([1, BAG], fp32)
            nc.sync.dma_start(brow[:], brow_in[:, :])
            if has_bias:
                bias_bc = constp.tile([TILE, BAG], fp32)
                nc.sync.dma_start(bias_bc[:], bb_in[:, :])

            stats_a = sps_a_p.tile([1, BAG], fp32)
            stats_b = sps_b_p.tile([1, BAG], fp32)
            agg_big = agg_p.tile([TILE, ng * BAG], fp32)

            # ---------------- phase 1: streamed quads (4 tiles each) -------
            QT = 4                  # tiles per quad
            assert t0 % QT == 0
            qpw = t0 // QT          # quads per window
            nq = nt // QT
            WID = QT * BAG          # 512

            import concourse.bass as bass_mod
            wps_tiles = {}
            xa = xb = None
            prev = None
            for q in range(nq + 1):
                if q < nq:
                    w, jq = divmod(q, qpw)
                    if jq == 0:
                        if use_bf16h:
                            xa = xa_p.tile([128, 2 * gi], bf16, tag="xa")
                            nc.sync.dma_start(
                                xa[:, 0:gi], xTh[0:128, w * gi:(w + 1) * gi])
                            nc.sync.dma_start(
                                xa[:, gi:2 * gi], xTl[0:128, w * gi:(w + 1) * gi])
                            xb = xb_p.tile([128, 2 * gi], bf16, tag="xb")
                            nc.sync.dma_start(
                                xb[:, 0:gi], xTh[128:256, w * gi:(w + 1) * gi])
                            nc.sync.dma_start(
                                xb[:, gi:2 * gi], xTl[128:256, w * gi:(w + 1) * gi])
                        else:
                            xa = xa_p.tile([128, gi], mmdt)
                            nc.sync.dma_start(xa[:], xT[0:128, w * gi:(w + 1) * gi])
                            xb = xb_p.tile([128, gi], mmdt)
                            nc.sync.dma_start(xb[:], xT[128:256, w * gi:(w + 1) * gi])
                        wt_ps = wps_p.tile([TILE, BAG], fp32)
                        wps_tiles[w] = wt_ps
                        nc.tensor.matmul(wt_ps[:], zeros_S[:], zeros_S[:, 0:BAG],
                                         start=True, stop=False)
                    hps = hps_p.tile([TILE, WID], fp32)
                    for j in range(QT):
                        c0 = (jq * QT + j) * 128
                        o = (j * BAG, (j + 1) * BAG)
                        if use_bf16h:
                            nc.tensor.matmul(hps[:, o[0]:o[1]],
                                             xa[:, c0:c0 + 128], wt0h[:],
                                             start=True, stop=False)
                            nc.tensor.matmul(hps[:, o[0]:o[1]],
                                             xa[:, c0:c0 + 128], wt0l[:],
                                             start=False, stop=False)
                            nc.tensor.matmul(hps[:, o[0]:o[1]],
                                             xb[:, c0:c0 + 128], wt1h[:],
                                             start=False, stop=False)
                            nc.tensor.matmul(hps[:, o[0]:o[1]],
                                             xb[:, c0:c0 + 128], wt1l[:],
                                             start=False, stop=False)
                            nc.tensor.matmul(hps[:, o[0]:o[1]],
                                             xa[:, gi + c0:gi + c0 + 128], wt0h[:],
                                             start=False, stop=False)
                            nc.tensor.matmul(hps[:, o[0]:o[1]],
                                             xb[:, gi + c0:gi + c0 + 128], wt1h[:],
                                             start=False, stop=True)
                        else:
                            nc.tensor.matmul(hps[:, o[0]:o[1]],
                                             xa[:, c0:c0 + 128], wt0[:],
                                             start=True, stop=False)
                            nc.tensor.matmul(hps[:, o[0]:o[1]],
                                             xb[:, c0:c0 + 128], wt1[:],
                                             start=False, stop=True)
                    if use_bf16seg:
                        hsb = hsb_p.tile([TILE, WID], bf16, tag="hsb_hi")
                        hlo = hsb_p.tile([TILE, WID], bf16, tag="hsb_lo")
                        nc.scalar.activation(hsb[:], hps[:], AFT.Relu)
                        nc.vector.scalar_tensor_tensor(
                            hlo[:], hps[:], 0.0, hsb[:], AOT.max, AOT.subtract)
                    else:
                        hlo = None
                        hsb = hsb_p.tile([TILE, WID], mmdt)
                    if use_bf16seg:
                        pass
                    elif has_bias:
                        bias4 = bass_mod.AP(
                            tensor=bias_bc.tensor, offset=bias_bc.offset,
                            ap=[bias_bc.ap[0], [0, QT], bias_bc.ap[1]])
                        nc.vector.tensor_tensor(
                            hsb[:].rearrange("p (a b) -> p a b", a=QT),
                            hps[:].rearrange("p (a b) -> p a b", a=QT),
                            bias4, AOT.add)
                        nc.vector.tensor_scalar_max(hsb[:], hsb[:], 0.0)
                    else:
                        if q % 2 == 0:
                            nc.vector.tensor_scalar_max(hsb[:], hps[:], 0.0)
                        else:
                            nc.scalar.activation(hsb[:], hps[:], AFT.Relu)
                    # wide selector: S[p, a*128 + f] = (iota[f] == slots[p, t0q+a])
                    S = s_p.tile([TILE, WID], segdt)
                    scol = slots_sb[:, q * QT:(q + 1) * QT]
                    srep = bass_mod.AP(tensor=scol.tensor, offset=scol.offset,
                                       ap=[scol.ap[0], scol.ap[1], [0, BAG]])
                    ibase = iota_sb[:]
                    irep = bass_mod.AP(tensor=ibase.tensor, offset=ibase.offset,
                                       ap=[ibase.ap[0], [0, QT], ibase.ap[1]])
                    nc.vector.tensor_tensor(
                        S[:].rearrange("p (a b) -> p a b", a=QT),
                        irep, srep, AOT.is_equal)
                    cur = (q, S, hsb, hlo, w, jq == qpw - 1)
                else:
                    cur = None
                if prev is not None:
                    pq, pS, phsb, phlo, pw, plast = prev
                    for j in range(QT):
                        last = plast and j == QT - 1
                        nc.tensor.matmul(wps_tiles[pw][:],
                                         pS[:, j * BAG:(j + 1) * BAG],
                                         phsb[:, j * BAG:(j + 1) * BAG],
                                         start=False,
                                         stop=(last and phlo is None))
                        if phlo is not None:
                            nc.tensor.matmul(wps_tiles[pw][:],
                                             pS[:, j * BAG:(j + 1) * BAG],
                                             phlo[:, j * BAG:(j + 1) * BAG],
                                             start=False, stop=last)
                    if plast:
                        aggw = agg_big[:, pw * BAG:(pw + 1) * BAG]
                        nc.scalar.activation(aggw, wps_tiles[pw][:], AFT.Copy,
                                             scale=recip_sb[:, pw:pw + 1])
                        a2 = agg2_p.tile([TILE, BAG], fp32)
                        nc.scalar.square(a2[:], aggw)
                        nc.tensor.matmul(stats_a[:], mask_sb[:, pw:pw + 1], aggw,
                                         start=(pw == 0), stop=(pw == ng - 1))
                        nc.tensor.matmul(stats_b[:], mask_sb[:, pw:pw + 1], a2[:],
                                         start=(pw == 0), stop=(pw == ng - 1))
                        del wps_tiles[pw]
                prev = cur

            # ---------------- stats all-reduce + params ----------------
            stats_sb = small_p.tile([1, 2 * BAG], fp32)
            nc.vector.tensor_copy(stats_sb[0:1, 0:BAG], stats_a[:])
            nc.vector.tensor_copy(stats_sb[0:1, BAG:2 * BAG], stats_b[:])
            cc_in = dram_p.tile([1, 2 * BAG], fp32)
            cc_out = dram_p.tile([1, 2 * BAG], fp32)
            nc.sync.dma_start(cc_in[:], stats_sb[:])
            nc.gpsimd.collective_compute(
                "AllReduce", AOT.add,
                replica_groups=[list(range(N_CORES))],
                ins=[cc_in.opt()], outs=[cc_out.opt()])
            gstats = small_p.tile([1, 2 * BAG], fp32)
            nc.sync.dma_start(gstats[:], cc_out[:])

            inv_nb = 1.0 / float(n_bags)
            mean = small_p.tile([1, BAG], fp32)
            nc.vector.tensor_scalar_mul(mean[:], gstats[0:1, 0:BAG], inv_nb)
            ex2 = small_p.tile([1, BAG], fp32)
            nc.vector.tensor_scalar_mul(ex2[:], gstats[0:1, BAG:2 * BAG], inv_nb)
            m2 = small_p.tile([1, BAG], fp32)
            nc.vector.tensor_tensor(m2[:], mean[:], mean[:], AOT.mult)
            vareps = small_p.tile([1, BAG], fp32)
            nc.vector.tensor_tensor(vareps[:], ex2[:], m2[:], AOT.subtract)
            nc.vector.tensor_scalar_add(vareps[:], vareps[:], EPS)
            rec = small_p.tile([1, BAG], fp32)
            nc.vector.reciprocal(rec[:], vareps[:])
            inv = small_p.tile([1, BAG], fp32)
            nc.scalar.sqrt(inv[:], rec[:])
            ab_row = small_p.tile([1, 2 * BAG], fp32)
            nc.vector.tensor_tensor(ab_row[0:1, 0:BAG], inv[:], grow[:], AOT.mult)
            mA = small_p.tile([1, BAG], fp32)
            nc.vector.tensor_tensor(mA[:], mean[:], ab_row[0:1, 0:BAG], AOT.mult)
            nc.vector.tensor_tensor(ab_row[0:1, BAG:2 * BAG], brow[:], mA[:],
                                    AOT.subtract)
            ab_ps = ab_p.tile([TILE, 2 * BAG], fp32)
            nc.tensor.matmul(ab_ps[:], ones_row[:], ab_row[:], start=True, stop=True)
            ab_sb = constp.tile([TILE, 2 * BAG], fp32)
            nc.vector.tensor_copy(ab_sb[:], ab_ps[:])

            # ---------------- phase 2: normalize + store ----------------
            a_col = ab_sb[:, 0:BAG]
            b_col = ab_sb[:, BAG:2 * BAG]
            w2 = 0
            while w2 < ng:
                nw = min(4, ng - w2)
                wid2 = nw * BAG
                arep = bass_mod.AP(tensor=a_col.tensor, offset=a_col.offset,
                                   ap=[a_col.ap[0], [0, nw], a_col.ap[1]])
                brep = bass_mod.AP(tensor=b_col.tensor, offset=b_col.offset,
                                   ap=[b_col.ap[0], [0, nw], b_col.ap[1]])
                ot = out_p.tile([TILE, 4 * BAG], fp32)
                src = agg_big[:, w2 * BAG:(w2 + nw) * BAG]
                nc.vector.tensor_tensor(
                    ot[:, 0:wid2].rearrange("p (a b) -> p a b", a=nw),
                    src.rearrange("p (a b) -> p a b", a=nw), arep, AOT.mult)
                nc.vector.tensor_tensor(
                    ot[:, 0:wid2].rearrange("p (a b) -> p a b", a=nw),
                    ot[:, 0:wid2].rearrange("p (a b) -> p a b", a=nw),
                    brep, AOT.add)
                # out rows for nw windows are contiguous: [w2*128, (w2+nw)*128)
                nc.sync.dma_start(
                    out[w2 * TILE:(w2 + nw) * TILE, :].rearrange(
                        "(a p) b -> p a b", p=TILE),
                    ot[:, 0:wid2].rearrange("p (a b) -> p a b", a=nw))
                w2 += nw

    nc.compile()
    return nc


# ----------------------------------------------------------------------------
# Entry point
# ----------------------------------------------------------------------------

def kernel(**inputs):
    global LAST_RESULTS
    from concourse.bass_utils import run_bass_kernel_spmd

    x = np.asarray(inputs["x"], dtype=np.float32)
    W = np.asarray(inputs["W"], dtype=np.float32)
    b = np.asarray(inputs["b"], dtype=np.float32)
    gamma = np.asarray(inputs["gamma"], dtype=np.float32)
    beta = np.asarray(inputs["beta"], dtype=np.float32)
    seg_ids = np.asarray(inputs["seg_ids"]).astype(np.int64)
    bags_len = np.asarray(inputs["bags_len"]).astype(np.int64)

    import os
    bf_mode = int(os.environ.get("KERNEL_BF16H", "4"))
    has_bias = bool(np.any(b != 0))
    if bf_mode >= 4 and has_bias:
        bf_mode = 3

    if bf_mode >= 4:
        plan, ng, in_maps, n_bags = _host_prep_t(
            x, W, gamma, beta, seg_ids, bags_len)
        key = (4, ng, n_bags)
        if key not in _NC_CACHE:
            _NC_CACHE[key] = _build_nc_t(ng, n_bags)
        nc = _NC_CACHE[key]
        res = run_bass_kernel_spmd(nc, in_maps, core_ids=list(range(N_CORES)))
        LAST_RESULTS = res
        out_full = np.zeros((n_bags, BAG), dtype=np.float32)
        for c in range(N_CORES):
            oc = np.asarray(res.results[c]["outT"]).astype(np.float32)
            for g, (p, e, fb, lbx) in enumerate(plan[c]["groups"]):
                ns = lbx - fb
                out_full[fb:lbx] = oc[:, g * SLOT_T: g * SLOT_T + ns].T
        return out_full

    plan, t0, ng, in_maps, n_bags = _host_prep(
        x, W, b, gamma, beta, seg_ids, bags_len, bf_mode)

    use_f32r = os.environ.get("KERNEL_F32R", "0") == "1"
    use_bf16h = 1 <= bf_mode <= 2
    use_bf16seg = bf_mode == 2 and not has_bias
    key = (ng, t0, n_bags, has_bias, use_f32r, bf_mode)
    if key not in _NC_CACHE:
        if bf_mode >= 3:
            _NC_CACHE[key] = _build_nc_pure(ng, t0, n_bags, has_bias)
        else:
            _NC_CACHE[key] = _build_nc(ng, t0, n_bags, has_bias,
                                       use_f32r=use_f32r, use_bf16h=use_bf16h,
                                       use_bf16seg=use_bf16seg)
    nc = _NC_CACHE[key]

    res = run_bass_kernel_spmd(nc, in_maps, core_ids=list(range(N_CORES)))
    LAST_RESULTS = res

    out_full = np.zeros((n_bags, BAG), dtype=np.float32)
    for c in range(N_CORES):
        oc = res.results[c]["out"]
        for g, (p, e, fb, lbx) in enumerate(plan[c]["groups"]):
            ns = lbx - fb
            out_full[fb:lbx] = oc[g * TILE: g * TILE + ns]
    return out_full


# revision 24
# speedup vs baseline: 1.1127x; 1.1127x over previous
"""Trainium2 Bass kernel for nn_BagInput (segment_reduce + linear/relu + BatchNorm).

Computation (matches the reference):
    h   = relu(x @ W.T + b)                      [N_items, 128]
    agg = segment_mean(h, seg_ids, NB)           [NB, 128]   (empty bags -> 0)
    out = (agg - mean) * rsqrt(var + eps) * gamma + beta   (batch stats over bags)

Strategy (8 NeuronCores, data-parallel over items, bag-aligned shards):
  - Host: shard items at bag boundaries; per core, pack items into groups of
    T0=16 128-item tiles, padding each group so that group boundaries fall on
    bag boundaries (~0.5% padding). One group == one "window" of <=128 bags.
  - Host: pre-transpose x to feature-major (xT) so the per-tile [feat, item]
    blocks DMA directly as matmul stationary operands. x is cast to bf16
    (mode 3, default): the rel-err budget (2e-2) dwarfs bf16 rounding
    (~0.5%), and it halves HBM traffic vs hi/lo-split f32 emulation.
  - Device per 128-item tile:
      h_psum = xT0_t.T @ WT0 + xT1_t.T @ WT1      (PE, K=256 in 2 chunks)
      h_sbuf = relu(h_psum) -> bf16               (DVE / ACT alternating)
      S      = (iota_row == slot_ids_t)           (0/1 selector, DVE, bf16)
      wps_w += S.T @ h_sbuf                       (PE; window accumulate in PSUM)
  - Window drain: agg = wps * (1/cnt)  (per-partition scalar); accumulate BN
    partial stats via mask.T @ [agg, agg^2] matmuls into PSUM.
  - AllReduce (8 cores) of the [1,256] stats; compute A = gamma*rsqrt(var+eps),
    B = beta - mean*A; broadcast via ones-matmul; out = agg*A + B; DMA out.
  - Host: gather per-(window,slot) rows back to global bag order.
"""

import numpy as np

N_CORES = 8
TILE = 128
FEAT = 256
BAG = 128
EPS = 1e-5

_NC_CACHE = {}
LAST_RESULTS = None  # BassKernelResults of the most recent run (for profiling)


# ----------------------------------------------------------------------------
# Host-side planning
# ----------------------------------------------------------------------------

def _plan_cores(seg_ids, n_bags, t0):
    """Split items/bags across cores at bag boundaries; pack groups of t0
    tiles per core such that each group covers whole bags (<=128 bags)."""
    gi = t0 * TILE
    n = seg_ids.shape[0]
    cuts = [0]
    bag_cuts = [0]
    for c in range(1, N_CORES):
        tgt = (n * c) // N_CORES
        bb = int(seg_ids[tgt])
        cut = int(np.searchsorted(seg_ids, bb, side="left"))
        cuts.append(cut)
        bag_cuts.append(bb)
    cuts.append(n)
    bag_cuts.append(n_bags)
    for c in range(N_CORES):
        if cuts[c + 1] <= cuts[c]:
            raise ValueError("degenerate core split")

    cores = []
    for c in range(N_CORES):
        i0, i1 = cuts[c], cuts[c + 1]
        b0, b1 = bag_cuts[c], bag_cuts[c + 1]
        seg = seg_ids[i0:i1]
        nloc = i1 - i0
        groups = []
        p = 0
        fb = b0
        while p < nloc:
            if p + gi >= nloc:
                e = nloc
                lbx = b1
            else:
                e = int(np.searchsorted(seg, seg[p + gi], side="left"))
                if e <= p:
                    raise ValueError("single bag larger than group size")
                lbx = int(seg[e - 1]) + 1
            if lbx - fb > TILE:
                raise ValueError(f"window spans {lbx - fb} bags > {TILE}")
            groups.append((p, e, fb, lbx))
            fb = lbx
            p = e
        cores.append(dict(i0=i0, i1=i1, b0=b0, b1=b1, groups=groups))
    return cores


def _host_prep(x, W, b, gamma, beta, seg_ids, bags_len, bf_mode):
    n_bags = bags_len.shape[0]
    plan = None
    for t0 in (16, 8, 4):
        try:
            plan = _plan_cores(seg_ids, n_bags, t0)
            break
        except ValueError:
            continue
    if plan is None:
        raise ValueError("could not plan groups")
    gi = t0 * TILE

    ng = max(len(c["groups"]) for c in plan)
    nt = ng * t0
    npad = ng * gi

    cnt = np.maximum(bags_len, 1).astype(np.float32)
    recip_all = 1.0 / cnt

    import ml_dtypes
    bf = ml_dtypes.bfloat16

    in_maps = []
    for c in range(N_CORES):
        info = plan[c]
        i0 = info["i0"]
        seg = seg_ids[i0:info["i1"]]
        groups = info["groups"]

        idx = np.full(npad, -1, dtype=np.int64)
        slots = np.full(npad, 255.0, dtype=np.float32)
        recip = np.ones((ng, TILE), dtype=np.float32)
        mask = np.zeros((ng, TILE), dtype=np.float32)
        for g, (p, e, fb, lbx) in enumerate(groups):
            m = e - p
            idx[g * gi: g * gi + m] = i0 + p + np.arange(m)
            slots[g * gi: g * gi + m] = (seg[p:e] - fb).astype(np.float32)
            ns = lbx - fb
            recip[g, :ns] = recip_all[fb:lbx]
            mask[g, :ns] = 1.0

        xp = np.zeros((npad, FEAT), dtype=np.float32)
        valid = idx >= 0
        xp[valid] = x[idx[valid]]
        xT = np.ascontiguousarray(xp.T)
        del xp

        common = {
            "recip": np.ascontiguousarray(recip.T),
            "mask": np.ascontiguousarray(mask.T),
            "gamma_row": np.ascontiguousarray(gamma.reshape(1, BAG)),
            "beta_row": np.ascontiguousarray(beta.reshape(1, BAG)),
            "bias_bc": np.ascontiguousarray(
                np.tile(b.reshape(1, BAG), (TILE, 1))),
        }
        if bf_mode >= 3:
            # pure bf16: single-precision x and W, halve HBM traffic
            in_maps.append({
                "xTh": np.ascontiguousarray(xT.astype(bf)),
                "WTh": np.ascontiguousarray(W.T.astype(bf)),
                "slots": np.ascontiguousarray(slots.reshape(nt, TILE).T.astype(bf)),
                "iota": np.ascontiguousarray(
                    np.tile(np.arange(TILE, dtype=np.float32), (TILE, 1)).astype(bf)),
                **common,
            })
        elif bf_mode >= 1:
            xTh = xT.astype(bf)
            xTl = (xT - xTh.astype(np.float32)).astype(bf)
            WTf = np.ascontiguousarray(W.T)
            WTh = WTf.astype(bf)
            WTl = (WTf - WTh.astype(np.float32)).astype(bf)
            in_maps.append({
                "xTh": np.ascontiguousarray(xTh),
                "xTl": np.ascontiguousarray(xTl),
                "WTh": np.ascontiguousarray(WTh),
                "WTl": np.ascontiguousarray(WTl),
                "slots": np.ascontiguousarray(slots.reshape(nt, TILE).T.astype(bf)),
                "iota": np.ascontiguousarray(
                    np.tile(np.arange(TILE, dtype=np.float32), (TILE, 1)).astype(bf)),
                **common,
            })
        else:
            in_maps.append({
                "xT": xT,
                "slots": np.ascontiguousarray(slots.reshape(nt, TILE).T),
                "WT": np.ascontiguousarray(W.T),
                "iota": np.ascontiguousarray(
                    np.tile(np.arange(TILE, dtype=np.float32), (TILE, 1))),
                **common,
            })
    return plan, t0, ng, in_maps, n_bags


# ----------------------------------------------------------------------------
# Mode 4: transposed seg-matmul, 64-slot windows, recip folded into x
# ----------------------------------------------------------------------------

T0_T = 8              # tiles per window (1024 items)
SLOT_T = 64           # one-hot width (<=64 bags per window)


def _plan_cores_t(seg_ids, n_bags):
    """Core split at bag boundaries; per core, windows of <=T0_T*128 items
    AND <=SLOT_T bags, each window covering whole bags."""
    gi = T0_T * TILE
    n = seg_ids.shape[0]
    cuts = [0]
    bag_cuts = [0]
    for c in range(1, N_CORES):
        tgt = (n * c) // N_CORES
        bb = int(seg_ids[tgt])
        cut = int(np.searchsorted(seg_ids, bb, side="left"))
        cuts.append(cut)
        bag_cuts.append(bb)
    cuts.append(n)
    bag_cuts.append(n_bags)
    for c in range(N_CORES):
        if cuts[c + 1] <= cuts[c]:
            raise ValueError("degenerate core split")

    cores = []
    for c in range(N_CORES):
        i0, i1 = cuts[c], cuts[c + 1]
        b0, b1 = bag_cuts[c], bag_cuts[c + 1]
        seg = seg_ids[i0:i1]
        nloc = i1 - i0
        groups = []
        p = 0
        fb = b0
        while p < nloc:
            if p + gi >= nloc:
                e = nloc
                lbx = b1
            else:
                e = int(np.searchsorted(seg, seg[p + gi], side="left"))
                if e <= p:
                    raise ValueError("single bag larger than window")
                lbx = int(seg[e - 1]) + 1
            if lbx - fb > SLOT_T:
                lbx = fb + SLOT_T
                e = int(np.searchsorted(seg, lbx, side="left"))
                if e <= p:
                    raise ValueError("bag-capped window is empty")
            groups.append((p, e, fb, lbx))
            fb = lbx
            p = e
        cores.append(dict(i0=i0, i1=i1, b0=b0, b1=b1, groups=groups))
    return cores


def _host_prep_t(x, W, gamma, beta, seg_ids, bags_len):
    n_bags = bags_len.shape[0]
    plan = _plan_cores_t(seg_ids, n_bags)
    gi = T0_T * TILE

    ng = max(len(c["groups"]) for c in plan)
    if ng % 2:
        ng += 1
    nt = ng * T0_T
    npad = ng * gi

    cnt = np.maximum(bags_len, 1).astype(np.float32)
    recip_all = (1.0 / cnt).astype(np.float32)
    xs = x * recip_all[np.asarray(seg_ids)][:, None]

    import ml_dtypes
    bf = ml_dtypes.bfloat16

    WT_bf = np.ascontiguousarray(W.T.astype(bf))
    iota64 = np.ascontiguousarray(
        np.tile(np.arange(SLOT_T, dtype=np.float32), (TILE, 1)).astype(bf))

    in_maps = []
    for c in range(N_CORES):
        info = plan[c]
        i0 = info["i0"]
        seg = seg_ids[i0:info["i1"]]
        groups = info["groups"]

        idx = np.full(npad, -1, dtype=np.int64)
        slots = np.full(npad, 255.0, dtype=np.float32)
        for g, (p, e, fb, lbx) in enumerate(groups):
            m = e - p
            idx[g * gi: g * gi + m] = i0 + p + np.arange(m)
            slots[g * gi: g * gi + m] = (seg[p:e] - fb).astype(np.float32)

        xp = np.zeros((npad, FEAT), dtype=np.float32)
        valid = idx >= 0
        xp[valid] = xs[idx[valid]]
        xT = np.ascontiguousarray(xp.T.astype(bf))
        del xp

        in_maps.append({
            "xT": xT,
            "WT": WT_bf,
            "slots": np.ascontiguousarray(slots.reshape(nt, TILE).T.astype(bf)),
            "iota64": iota64,
            "gamma_col": np.ascontiguousarray(gamma.reshape(BAG, 1)),
            "beta_col": np.ascontiguousarray(beta.reshape(BAG, 1)),
        })
    return plan, ng, in_maps, n_bags


def _build_nc_t(ng, n_bags):
    import os
    use_ttr = os.environ.get("KERNEL_TTR", "0") == "1"
    use_cc = os.environ.get("KERNEL_NOCC", "0") != "1"
    import concourse.bacc as bacc
    import concourse.tile as tile
    import concourse.mybir as mybir
    import concourse.bass as bass_mod

    fp32 = mybir.dt.float32
    bf16 = mybir.dt.bfloat16
    AOT = mybir.AluOpType
    AFT = mybir.ActivationFunctionType

    gi = T0_T * TILE
    nt = ng * T0_T
    npad = ng * gi
    np2 = ng // 2
    SW = T0_T * SLOT_T          # selector cols per window (512)

    nc = bacc.Bacc("TRN2", target_bir_lowering=False, debug=False,
                   enable_asserts=False, num_devices=N_CORES)
    xT = nc.dram_tensor("xT", [FEAT, npad], bf16, kind="ExternalInput")
    WT_in = nc.dram_tensor("WT", [FEAT, BAG], bf16, kind="ExternalInput")
    slots = nc.dram_tensor("slots", [TILE, nt], bf16, kind="ExternalInput")
    iota_in = nc.dram_tensor("iota64", [TILE, SLOT_T], bf16, kind="ExternalInput")
    gcol_in = nc.dram_tensor("gamma_col", [BAG, 1], fp32, kind="ExternalInput")
    bcol_in = nc.dram_tensor("beta_col", [BAG, 1], fp32, kind="ExternalInput")
    outT = nc.dram_tensor("outT", [TILE, np2 * TILE], bf16, kind="ExternalOutput")

    with tile.TileContext(nc) as tc:
        with tc.tile_pool(name="const", bufs=1) as constp, \
             tc.tile_pool(name="xa", bufs=8) as xa_p, \
             tc.tile_pool(name="xb", bufs=8) as xb_p, \
             tc.tile_pool(name="hsb", bufs=4) as hsb_p, \
             tc.tile_pool(name="Sp", bufs=4) as s_p, \
             tc.tile_pool(name="agg", bufs=1) as agg_p, \
             tc.tile_pool(name="sq", bufs=2) as sq_p, \
             tc.tile_pool(name="stat", bufs=4) as stat_p, \
             tc.tile_pool(name="outp", bufs=2) as out_p, \
             tc.tile_pool(name="small", bufs=1) as small_p, \
             tc.tile_pool(name="hps", bufs=3, space="PSUM") as hps_p, \
             tc.tile_pool(name="wpsp", bufs=2, space="PSUM") as wps_p, \
             tc.tile_pool(name="dram", bufs=1, space="DRAM") as dram_p:

            wt0 = constp.tile([128, BAG], bf16)
            nc.sync.dma_start(wt0[:], WT_in[0:128, :])
            wt1 = constp.tile([128, BAG], bf16)
            nc.sync.dma_start(wt1[:], WT_in[128:256, :])
            iota_sb = constp.tile([TILE, SLOT_T], bf16)
            nc.sync.dma_start(iota_sb[:], iota_in[:, :])
            slots_sb = constp.tile([TILE, nt], bf16)
            nc.sync.dma_start(slots_sb[:], slots[:, :])
            gcol = constp.tile([BAG, 1], fp32)
            nc.sync.dma_start(gcol[:], gcol_in[:, :])
            bcol = constp.tile([BAG, 1], fp32)
            nc.sync.dma_start(bcol[:], bcol_in[:, :])
            zcol = constp.tile([TILE, 1], fp32)
            nc.vector.memset(zcol[:], 0.0)

            agg_big = agg_p.tile([TILE, np2 * BAG], fp32)
            if not use_ttr:
                sa_tab = constp.tile([TILE, np2], fp32)
                sb_tab = constp.tile([TILE, np2], fp32)

            wps_tiles = {}
            prev = None
            sa_prev = 0.0
            sb_prev = 0.0
            for q in range(ng + 1):
                if q < ng:
                    w = q
                    xa = xa_p.tile([128, gi], bf16, tag="xa")
                    nc.sync.dma_start(xa[:], xT[0:128, w * gi:(w + 1) * gi])
                    xb = xb_p.tile([128, gi], bf16, tag="xb")
                    nc.sync.dma_start(xb[:], xT[128:256, w * gi:(w + 1) * gi])
                    if w % 2 == 0:
                        # full-bank tile so the pair's accumulation groups and
                        # the drain never share a PSUM bank with another pair
                        wt_full = wps_p.tile([TILE, 512], fp32)
                        wps_tiles[w // 2] = wt_full
                    HW2 = T0_T * BAG // 2            # 512 = one PSUM bank
                    hpsa = hps_p.tile([TILE, HW2], fp32, tag="hpsa")
                    hpsb = hps_p.tile([TILE, HW2], fp32, tag="hpsb")
                    for j in range(T0_T):
                        hp = hpsa if j < 4 else hpsb
                        o = ((j % 4) * BAG, (j % 4 + 1) * BAG)
                        nc.tensor.matmul(hp[:, o[0]:o[1]],
                                         xa[:, j * 128:(j + 1) * 128], wt0[:],
                                         start=True, stop=False)
                        nc.tensor.matmul(hp[:, o[0]:o[1]],
                                         xb[:, j * 128:(j + 1) * 128], wt1[:],
                                         start=False, stop=True)
                    hsb = hsb_p.tile([TILE, T0_T * BAG], bf16)
                    nc.scalar.activation(hsb[:, 0:HW2], hpsa[:], AFT.Relu)
                    nc.scalar.activation(hsb[:, HW2:2 * HW2], hpsb[:], AFT.Relu)
                    # selector: S[p, a*64 + f] = (iota64[f] == slots[p, 8w+a])
                    S = s_p.tile([TILE, SW], bf16)
                    scol = slots_sb[:, w * T0_T:(w + 1) * T0_T]
                    srep = bass_mod.AP(tensor=scol.tensor, offset=scol.offset,
                                       ap=[scol.ap[0], scol.ap[1], [0, SLOT_T]])
                    ibase = iota_sb[:]
                    irep = bass_mod.AP(tensor=ibase.tensor, offset=ibase.offset,
                                       ap=[ibase.ap[0], [0, T0_T], ibase.ap[1]])
                    nc.vector.tensor_tensor(
                        S[:].rearrange("p (a b) -> p a b", a=T0_T),
                        irep, srep, AOT.is_equal)
                    cur = (w, hsb, S)
                else:
                    cur = None
                if prev is not None:
                    pw, phsb, pS = prev
                    # windows of a pair land at 512B-aligned cols 0 and 128
                    h0 = (pw % 2) * BAG
                    k = pw // 2
                    wt_ps = wps_tiles[k]
                    for j in range(T0_T):
                        nc.tensor.matmul(wt_ps[:, h0:h0 + SLOT_T],
                                         phsb[:, j * BAG:(j + 1) * BAG],
                                         pS[:, j * SLOT_T:(j + 1) * SLOT_T],
                                         start=(j == 0), stop=(j == T0_T - 1))
                    if pw % 2 == 1:
                        aggsl = agg_big[:, k * BAG:(k + 1) * BAG]
                        zrep = bass_mod.AP(tensor=zcol.tensor, offset=zcol.offset,
                                           ap=[zcol.ap[0], [0, 2], [0, SLOT_T]])
                        wsrc = wt_ps[:]
                        wstr = bass_mod.AP(tensor=wsrc.tensor, offset=wsrc.offset,
                                           ap=[wsrc.ap[0], [BAG, 2], [1, SLOT_T]])
                        if use_ttr:
                            sa_new = stat_p.tile([TILE, 1], fp32, tag="sa")
                            nc.vector.tensor_tensor_reduce(
                                aggsl.rearrange("p (a b) -> p a b", a=2),
                                wstr, zrep, 1.0, sa_prev,
                                AOT.add, AOT.add, sa_new[:])
                            sq = sq_p.tile([TILE, BAG], fp32)
                            sb_new = stat_p.tile([TILE, 1], fp32, tag="sb")
                            nc.vector.tensor_tensor_reduce(
                                sq[:], aggsl, aggsl, 1.0, sb_prev,
                                AOT.mult, AOT.add, sb_new[:])
                            sa_prev = sa_new[:]
                            sb_prev = sb_new[:]
                        else:
                            nc.vector.scalar_tensor_tensor(
                                aggsl.rearrange("p (a b) -> p a b", a=2),
                                wstr, 1.0, zrep, AOT.mult, AOT.add,
                                accum_out=sa_tab[:, k:k + 1])
                            sq = sq_p.tile([TILE, BAG], fp32)
                            nc.vector.scalar_tensor_tensor(
                                sq[:], aggsl, 0.0, aggsl, AOT.add, AOT.mult,
                                accum_out=sb_tab[:, k:k + 1])
                        del wps_tiles[k]
                prev = cur

            # ---------------- stats all-reduce + params ----------------
            stats_sb = small_p.tile([TILE, 2], fp32)
            if use_ttr:
                nc.vector.tensor_copy(stats_sb[:, 0:1], sa_prev)
                nc.vector.tensor_copy(stats_sb[:, 1:2], sb_prev)
            else:
                nc.vector.tensor_reduce(stats_sb[:, 0:1], sa_tab[:],
                                        mybir.AxisListType.X, AOT.add)
                nc.vector.tensor_reduce(stats_sb[:, 1:2], sb_tab[:],
                                        mybir.AxisListType.X, AOT.add)
            gstats = small_p.tile([TILE, 2], fp32)
            if use_cc:
                # flat [1, 256] views so the CC reduces one contiguous row
                cc_in = dram_p.tile([1, 2 * BAG], fp32)
                cc_out = dram_p.tile([1, 2 * BAG], fp32)
                cc_in_t = cc_in[0:1, :].rearrange("a (p b) -> p a b", p=TILE)
                cc_out_t = cc_out[0:1, :].rearrange("a (p b) -> p a b", p=TILE)
                nc.sync.dma_start(
                    cc_in_t, stats_sb[:].rearrange("p (a b) -> p a b", a=1))
                nc.gpsimd.collective_compute(
                    "AllReduce", AOT.add,
                    replica_groups=[list(range(N_CORES))],
                    ins=[cc_in.opt()], outs=[cc_out.opt()])
                nc.sync.dma_start(
                    gstats[:].rearrange("p (a b) -> p a b", a=1), cc_out_t)
            else:
                nc.vector.tensor_copy(gstats[:], stats_sb[:])

            inv_nb = 1.0 / float(n_bags)
            mean = small_p.tile([TILE, 1], fp32)
            nc.vector.tensor_scalar_mul(mean[:], gstats[:, 0:1], inv_nb)
            ex2 = small_p.tile([TILE, 1], fp32)
            nc.vector.tensor_scalar_mul(ex2[:], gstats[:, 1:2], inv_nb)
            m2 = small_p.tile([TILE, 1], fp32)
            nc.vector.tensor_tensor(m2[:], mean[:], mean[:], AOT.mult)
            vareps = small_p.tile([TILE, 1], fp32)
            nc.vector.tensor_tensor(vareps[:], ex2[:], m2[:], AOT.subtract)
            nc.vector.tensor_scalar_add(vareps[:], vareps[:], EPS)
            rec = small_p.tile([TILE, 1], fp32)
            nc.vector.reciprocal(rec[:], vareps[:])
            inv = small_p.tile([TILE, 1], fp32)
            nc.scalar.sqrt(inv[:], rec[:])
            acol = small_p.tile([TILE, 1], fp32)
            nc.vector.tensor_tensor(acol[:], inv[:], gcol[:], AOT.mult)
            mA = small_p.tile([TILE, 1], fp32)
            nc.vector.tensor_tensor(mA[:], mean[:], acol[:], AOT.mult)
            bcol2 = small_p.tile([TILE, 1], fp32)
            nc.vector.tensor_tensor(bcol2[:], bcol[:], mA[:], AOT.subtract)

            # ---------------- phase 2: normalize + store ----------------
            CH = 1024
            tot = np2 * BAG
            w2 = 0
            while w2 < tot:
                cw = min(CH, tot - w2)
                brep = bass_mod.AP(tensor=bcol2.tensor, offset=bcol2.offset,
                                   ap=[bcol2.ap[0], [0, cw]])
                ot = out_p.tile([TILE, CH], bf16)
                nc.vector.scalar_tensor_tensor(
                    ot[:, 0:cw], agg_big[:, w2:w2 + cw], acol[:], brep,
                    AOT.mult, AOT.add)
                nc.sync.dma_start(outT[:, w2:w2 + cw], ot[:, 0:cw])
                w2 += cw

    nc.compile()
    return nc


# ----------------------------------------------------------------------------
# Device kernel (pure bf16, mode 3)
# ----------------------------------------------------------------------------

def _build_nc_pure(ng, t0, n_bags, has_bias):
    import concourse.bacc as bacc
    import concourse.tile as tile
    import concourse.mybir as mybir
    import concourse.bass as bass_mod

    fp32 = mybir.dt.float32
    bf16 = mybir.dt.bfloat16
    AOT = mybir.AluOpType
    AFT = mybir.ActivationFunctionType

    gi = t0 * TILE
    nt = ng * t0

    npad = ng * gi

    nc = bacc.Bacc("TRN2", target_bir_lowering=False, debug=False,
                   enable_asserts=False, num_devices=N_CORES)
    xTh = nc.dram_tensor("xTh", [FEAT, npad], bf16, kind="ExternalInput")
    WTh_in = nc.dram_tensor("WTh", [FEAT, BAG], bf16, kind="ExternalInput")
    slots = nc.dram_tensor("slots", [TILE, nt], bf16, kind="ExternalInput")
    recip = nc.dram_tensor("recip", [TILE, ng], fp32, kind="ExternalInput")
    mask = nc.dram_tensor("mask", [TILE, ng], fp32, kind="ExternalInput")
    iota_in = nc.dram_tensor("iota", [TILE, TILE], bf16, kind="ExternalInput")
    grow_in = nc.dram_tensor("gamma_row", [1, BAG], fp32, kind="ExternalInput")
    brow_in = nc.dram_tensor("beta_row", [1, BAG], fp32, kind="ExternalInput")
    bb_in = nc.dram_tensor("bias_bc", [TILE, BAG], fp32, kind="ExternalInput")
    out = nc.dram_tensor("out", [ng * TILE, BAG], fp32, kind="ExternalOutput")

    with tile.TileContext(nc) as tc:
        with tc.tile_pool(name="const", bufs=1) as constp, \
             tc.tile_pool(name="xa", bufs=5) as xa_p, \
             tc.tile_pool(name="xb", bufs=5) as xb_p, \
             tc.tile_pool(name="hsb", bufs=6) as hsb_p, \
             tc.tile_pool(name="Sp", bufs=6) as s_p, \
             tc.tile_pool(name="agg", bufs=1) as agg_p, \
             tc.tile_pool(name="agg2", bufs=2) as agg2_p, \
             tc.tile_pool(name="outp", bufs=2) as out_p, \
             tc.tile_pool(name="small", bufs=1) as small_p, \
             tc.tile_pool(name="hps", bufs=3, space="PSUM") as hps_p, \
             tc.tile_pool(name="wpsp", bufs=2, space="PSUM") as wps_p, \
             tc.tile_pool(name="spsa", bufs=1, space="PSUM") as sps_a_p, \
             tc.tile_pool(name="spsb", bufs=1, space="PSUM") as sps_b_p, \
             tc.tile_pool(name="abps", bufs=1, space="PSUM") as ab_p, \
             tc.tile_pool(name="dram", bufs=1, space="DRAM") as dram_p:

            wt0 = constp.tile([128, BAG], bf16)
            nc.sync.dma_start(wt0[:], WTh_in[0:128, :])
            wt1 = constp.tile([128, BAG], bf16)
            nc.sync.dma_start(wt1[:], WTh_in[128:256, :])
            iota_sb = constp.tile([TILE, TILE], bf16)
            nc.sync.dma_start(iota_sb[:], iota_in[:, :])
            recip_sb = constp.tile([TILE, ng], fp32)
            nc.sync.dma_start(recip_sb[:], recip[:, :])
            mask_sb = constp.tile([TILE, ng], fp32)
            nc.sync.dma_start(mask_sb[:], mask[:, :])
            slots_sb = constp.tile([TILE, nt], bf16)
            nc.sync.dma_start(slots_sb[:], slots[:, :])
            ones_row = constp.tile([1, TILE], fp32)
            nc.vector.memset(ones_row[:], 1.0)
            grow = constp.tile([1, BAG], fp32)
            nc.sync.dma_start(grow[:], grow_in[:, :])
            brow = constp.tile([1, BAG], fp32)
            nc.sync.dma_start(brow[:], brow_in[:, :])
            if has_bias:
                bias_bc = constp.tile([TILE, BAG], fp32)
                nc.sync.dma_start(bias_bc[:], bb_in[:, :])

            stats_a = sps_a_p.tile([1, BAG], fp32)
            stats_b = sps_b_p.tile([1, BAG], fp32)
            agg_big = agg_p.tile([TILE, ng * BAG], fp32)

            # ---------------- phase 1: streamed quads (4 tiles each) -------
            QT = 4                  # tiles per quad
            assert t0 % QT == 0
            qpw = t0 // QT          # quads per window
            nq = nt // QT
            WID = QT * BAG          # 512

            wps_tiles = {}
            xa = xb = None
            prev = None
            for q in range(nq + 1):
                if q < nq:
                    w, jq = divmod(q, qpw)
                    if jq == 0:
                        xa = xa_p.tile([128, gi], bf16, tag="xa")
                        nc.sync.dma_start(xa[:], xTh[0:128, w * gi:(w + 1) * gi])
                        xb = xb_p.tile([128, gi], bf16, tag="xb")
                        nc.sync.dma_start(xb[:], xTh[128:256, w * gi:(w + 1) * gi])
                        wt_ps = wps_p.tile([TILE, BAG], fp32)
                        wps_tiles[w] = wt_ps
                    hps = hps_p.tile([TILE, WID], fp32)
                    for j in range(QT):
                        c0 = (jq * QT + j) * 128
                        o = (j * BAG, (j + 1) * BAG)
                        nc.tensor.matmul(hps[:, o[0]:o[1]],
                                         xa[:, c0:c0 + 128], wt0[:],
                                         start=True, stop=False)
                        nc.tensor.matmul(hps[:, o[0]:o[1]],
                                         xb[:, c0:c0 + 128], wt1[:],
                                         start=False, stop=True)
                    hsb = hsb_p.tile([TILE, WID], bf16)
                    if has_bias:
                        bias4 = bass_mod.AP(
                            tensor=bias_bc.tensor, offset=bias_bc.offset,
                            ap=[bias_bc.ap[0], [0, QT], bias_bc.ap[1]])
                        nc.vector.tensor_tensor(
                            hsb[:].rearrange("p (a b) -> p a b", a=QT),
                            hps[:].rearrange("p (a b) -> p a b", a=QT),
                            bias4, AOT.add)
                        nc.vector.tensor_scalar_max(hsb[:], hsb[:], 0.0)
                    else:
                        nc.scalar.activation(hsb[:], hps[:], AFT.Relu)
                    # wide selector: S[p, a*128 + f] = (iota[f] == slots[p, t0q+a])
                    S = s_p.tile([TILE, WID], bf16)
                    scol = slots_sb[:, q * QT:(q + 1) * QT]
                    srep = bass_mod.AP(tensor=scol.tensor, offset=scol.offset,
                                       ap=[scol.ap[0], scol.ap[1], [0, BAG]])
                    ibase = iota_sb[:]
                    irep = bass_mod.AP(tensor=ibase.tensor, offset=ibase.offset,
                                       ap=[ibase.ap[0], [0, QT], ibase.ap[1]])
                    nc.vector.tensor_tensor(
                        S[:].rearrange("p (a b) -> p a b", a=QT),
                        irep, srep, AOT.is_equal)
                    cur = (q, S, hsb, w, jq == 0, jq == qpw - 1)
                else:
                    cur = None
                if prev is not None:
                    pq, pS, phsb, pw, pfirst, plast = prev
                    for j in range(QT):
                        nc.tensor.matmul(wps_tiles[pw][:],
                                         pS[:, j * BAG:(j + 1) * BAG],
                                         phsb[:, j * BAG:(j + 1) * BAG],
                                         start=(pfirst and j == 0),
                                         stop=(plast and j == QT - 1))
                    if plast:
                        aggw = agg_big[:, pw * BAG:(pw + 1) * BAG]
                        nc.scalar.activation(aggw, wps_tiles[pw][:], AFT.Copy,
                                             scale=recip_sb[:, pw:pw + 1])
                        a2 = agg2_p.tile([TILE, BAG], fp32)
                        nc.vector.tensor_tensor(a2[:], aggw, aggw, AOT.mult)
                        nc.tensor.matmul(stats_a[:], mask_sb[:, pw:pw + 1], aggw,
                                         start=(pw == 0), stop=(pw == ng - 1))
                        nc.tensor.matmul(stats_b[:], mask_sb[:, pw:pw + 1], a2[:],
                                         start=(pw == 0), stop=(pw == ng - 1))
                        del wps_tiles[pw]
                prev = cur

            # ---------------- stats all-reduce + params ----------------
            stats_sb = small_p.tile([1, 2 * BAG], fp32)
            nc.vector.tensor_copy(stats_sb[0:1, 0:BAG], stats_a[:])
            nc.vector.tensor_copy(stats_sb[0:1, BAG:2 * BAG], stats_b[:])
            cc_in = dram_p.tile([1, 2 * BAG], fp32)
            cc_out = dram_p.tile([1, 2 * BAG], fp32)
            nc.sync.dma_start(cc_in[:], stats_sb[:])
            nc.gpsimd.collective_compute(
                "AllReduce", AOT.add,
                replica_groups=[list(range(N_CORES))],
                ins=[cc_in.opt()], outs=[cc_out.opt()])
            gstats = small_p.tile([1, 2 * BAG], fp32)
            nc.sync.dma_start(gstats[:], cc_out[:])

            inv_nb = 1.0 / float(n_bags)
            mean = small_p.tile([1, BAG], fp32)
            nc.vector.tensor_scalar_mul(mean[:], gstats[0:1, 0:BAG], inv_nb)
            ex2 = small_p.tile([1, BAG], fp32)
            nc.vector.tensor_scalar_mul(ex2[:], gstats[0:1, BAG:2 * BAG], inv_nb)
            m2 = small_p.tile([1, BAG], fp32)
            nc.vector.tensor_tensor(m2[:], mean[:], mean[:], AOT.mult)
            vareps = small_p.tile([1, BAG], fp32)
            nc.vector.tensor_tensor(vareps[:], ex2[:], m2[:], AOT.subtract)
            nc.vector.tensor_scalar_add(vareps[:], vareps[:], EPS)
            rec = small_p.tile([1, BAG], fp32)
            nc.vector.reciprocal(rec[:], vareps[:])
            inv = small_p.tile([1, BAG], fp32)
            nc.scalar.sqrt(inv[:], rec[:])
            ab_row = small_p.tile([1, 2 * BAG], fp32)
            nc.vector.tensor_tensor(ab_row[0:1, 0:BAG], inv[:], grow[:], AOT.mult)
            mA = small_p.tile([1, BAG], fp32)
            nc.vector.tensor_tensor(mA[:], mean[:], ab_row[0:1, 0:BAG], AOT.mult)
            nc.vector.tensor_tensor(ab_row[0:1, BAG:2 * BAG], brow[:], mA[:],
                                    AOT.subtract)
            ab_ps = ab_p.tile([TILE, 2 * BAG], fp32)
            nc.tensor.matmul(ab_ps[:], ones_row[:], ab_row[:], start=True, stop=True)
            ab_sb = constp.tile([TILE, 2 * BAG], fp32)
            nc.vector.tensor_copy(ab_sb[:], ab_ps[:])

            # ---------------- phase 2: normalize + store ----------------
            a_col = ab_sb[:, 0:BAG]
            b_col = ab_sb[:, BAG:2 * BAG]
            w2 = 0
            while w2 < ng:
                nw = min(4, ng - w2)
                wid2 = nw * BAG
                arep = bass_mod.AP(tensor=a_col.tensor, offset=a_col.offset,
                                   ap=[a_col.ap[0], [0, nw], a_col.ap[1]])
                brep = bass_mod.AP(tensor=b_col.tensor, offset=b_col.offset,
                                   ap=[b_col.ap[0], [0, nw], b_col.ap[1]])
                ot = out_p.tile([TILE, 4 * BAG], fp32)
                src = agg_big[:, w2 * BAG:(w2 + nw) * BAG]
                nc.vector.tensor_tensor(
                    ot[:, 0:wid2].rearrange("p (a b) -> p a b", a=nw),
                    src.rearrange("p (a b) -> p a b", a=nw), arep, AOT.mult)
                nc.vector.tensor_tensor(
                    ot[:, 0:wid2].rearrange("p (a b) -> p a b", a=nw),
                    ot[:, 0:wid2].rearrange("p (a b) -> p a b", a=nw),
                    brep, AOT.add)
                # out rows for nw windows are contiguous: [w2*128, (w2+nw)*128)
                nc.sync.dma_start(
                    out[w2 * TILE:(w2 + nw) * TILE, :].rearrange(
                        "(a p) b -> p a b", p=TILE),
                    ot[:, 0:wid2].rearrange("p (a b) -> p a b", a=nw))
                w2 += nw

    nc.compile()
    return nc


# ----------------------------------------------------------------------------
# Device kernel (legacy modes 0-2: f32 / bf16 hi-lo split)
# ----------------------------------------------------------------------------

def _build_nc(ng, t0, n_bags, has_bias, relu_dve_mod=2, sbuild_dve_mod=4,
              use_f32r=False, use_bf16h=False, use_bf16seg=False):
    import concourse.bacc as bacc
    import concourse.tile as tile
    import concourse.mybir as mybir

    fp32 = mybir.dt.float32
    mmdt = mybir.dt.float32r if use_f32r else fp32
    bf16 = mybir.dt.bfloat16
    AOT = mybir.AluOpType
    AFT = mybir.ActivationFunctionType

    gi = t0 * TILE
    nt = ng * t0
    npad = ng * gi

    nc = bacc.Bacc("TRN2", target_bir_lowering=False, debug=False,
                   enable_asserts=False, num_devices=N_CORES)
    if use_bf16h:
        xTh = nc.dram_tensor("xTh", [FEAT, npad], bf16, kind="ExternalInput")
        xTl = nc.dram_tensor("xTl", [FEAT, npad], bf16, kind="ExternalInput")
        WTh_in = nc.dram_tensor("WTh", [FEAT, BAG], bf16, kind="ExternalInput")
        WTl_in = nc.dram_tensor("WTl", [FEAT, BAG], bf16, kind="ExternalInput")
    else:
        xT = nc.dram_tensor("xT", [FEAT, npad], mmdt, kind="ExternalInput")
        WT = nc.dram_tensor("WT", [FEAT, BAG], mmdt, kind="ExternalInput")
    sldt = bf16 if use_bf16h else fp32
    slots = nc.dram_tensor("slots", [TILE, nt], sldt, kind="ExternalInput")
    recip = nc.dram_tensor("recip", [TILE, ng], fp32, kind="ExternalInput")
    mask = nc.dram_tensor("mask", [TILE, ng], fp32, kind="ExternalInput")
    iota_in = nc.dram_tensor("iota", [TILE, TILE], sldt, kind="ExternalInput")
    grow_in = nc.dram_tensor("gamma_row", [1, BAG], fp32, kind="ExternalInput")
    brow_in = nc.dram_tensor("beta_row", [1, BAG], fp32, kind="ExternalInput")
    bb_in = nc.dram_tensor("bias_bc", [TILE, BAG], fp32, kind="ExternalInput")
    out = nc.dram_tensor("out", [ng * TILE, BAG], fp32, kind="ExternalOutput")

    with tile.TileContext(nc) as tc:
        with tc.tile_pool(name="const", bufs=1) as constp, \
             tc.tile_pool(name="xa", bufs=4) as xa_p, \
             tc.tile_pool(name="xb", bufs=4) as xb_p, \
             tc.tile_pool(name="hsb", bufs=6) as hsb_p, \
             tc.tile_pool(name="Sp", bufs=6) as s_p, \
             tc.tile_pool(name="agg", bufs=1) as agg_p, \
             tc.tile_pool(name="agg2", bufs=2) as agg2_p, \
             tc.tile_pool(name="outp", bufs=2) as out_p, \
             tc.tile_pool(name="small", bufs=1) as small_p, \
             tc.tile_pool(name="hps", bufs=3, space="PSUM") as hps_p, \
             tc.tile_pool(name="wpsp", bufs=2, space="PSUM") as wps_p, \
             tc.tile_pool(name="spsa", bufs=1, space="PSUM") as sps_a_p, \
             tc.tile_pool(name="spsb", bufs=1, space="PSUM") as sps_b_p, \
             tc.tile_pool(name="abps", bufs=1, space="PSUM") as ab_p, \
             tc.tile_pool(name="dram", bufs=1, space="DRAM") as dram_p:

            if use_bf16h:
                wt0h = constp.tile([128, BAG], bf16)
                nc.sync.dma_start(wt0h[:], WTh_in[0:128, :])
                wt1h = constp.tile([128, BAG], bf16)
                nc.sync.dma_start(wt1h[:], WTh_in[128:256, :])
                wt0l = constp.tile([128, BAG], bf16)
                nc.sync.dma_start(wt0l[:], WTl_in[0:128, :])
                wt1l = constp.tile([128, BAG], bf16)
                nc.sync.dma_start(wt1l[:], WTl_in[128:256, :])
            else:
                wt0 = constp.tile([128, BAG], mmdt)
                nc.sync.dma_start(wt0[:], WT[0:128, :])
                wt1 = constp.tile([128, BAG], mmdt)
                nc.sync.dma_start(wt1[:], WT[128:256, :])
            iota_sb = constp.tile([TILE, TILE], sldt)
            nc.sync.dma_start(iota_sb[:], iota_in[:, :])
            recip_sb = constp.tile([TILE, ng], fp32)
            nc.sync.dma_start(recip_sb[:], recip[:, :])
            mask_sb = constp.tile([TILE, ng], fp32)
            nc.sync.dma_start(mask_sb[:], mask[:, :])
            slots_sb = constp.tile([TILE, nt], sldt)
            nc.sync.dma_start(slots_sb[:], slots[:, :])
            segdt = bf16 if use_bf16seg else mmdt
            zeros_f32 = constp.tile([TILE, TILE], fp32)
            nc.vector.memset(zeros_f32[:], 0.0)
            if use_f32r or use_bf16seg:
                zeros_S = constp.tile([TILE, TILE], segdt)
                nc.vector.tensor_copy(zeros_S[:], zeros_f32[:])
            else:
                zeros_S = zeros_f32
            ones_row = constp.tile([1, TILE], fp32)
            nc.vector.memset(ones_row[:], 1.0)
            grow = constp.tile([1, BAG], fp32)
            nc.sync.dma_start(grow[:], grow_in[:, :])
            brow = constp.tile([1, BAG], fp32)
            nc.sync.dma_start(brow[:], brow_in[:, :])
            if has_bias:
                bias_bc = constp.tile([TILE, BAG], fp32)
                nc.sync.dma_start(bias_bc[:], bb_in[:, :])

            stats_a = sps_a_p.tile([1, BAG], fp32)
            stats_b = sps_b_p.tile([1, BAG], fp32)
            agg_big = agg_p.tile([TILE, ng * BAG], fp32)

            # ---------------- phase 1: streamed quads (4 tiles each) -------
            QT = 4                  # tiles per quad
            assert t0 % QT == 0
            qpw = t0 // QT          # quads per window
            nq = nt // QT
            WID = QT * BAG          # 512

            import concourse.bass as bass_mod
            wps_tiles = {}
            xa = xb = None
            prev = None
            for q in range(nq + 1):
                if q < nq:
                    w, jq = divmod(q, qpw)
                    if jq == 0:
                        if use_bf16h:
                            xa = xa_p.tile([128, 2 * gi], bf16, tag="xa")
                            nc.sync.dma_start(
                                xa[:, 0:gi], xTh[0:128, w * gi:(w + 1) * gi])
                            nc.sync.dma_start(
                                xa[:, gi:2 * gi], xTl[0:128, w * gi:(w + 1) * gi])
                            xb = xb_p.tile([128, 2 * gi], bf16, tag="xb")
                            nc.sync.dma_start(
                                xb[:, 0:gi], xTh[128:256, w * gi:(w + 1) * gi])
                            nc.sync.dma_start(
                                xb[:, gi:2 * gi], xTl[128:256, w * gi:(w + 1) * gi])
                        else:
                            xa = xa_p.tile([128, gi], mmdt)
                            nc.sync.dma_start(xa[:], xT[0:128, w * gi:(w + 1) * gi])
                            xb = xb_p.tile([128, gi], mmdt)
                            nc.sync.dma_start(xb[:], xT[128:256, w * gi:(w + 1) * gi])
                        wt_ps = wps_p.tile([TILE, BAG], fp32)
                        wps_tiles[w] = wt_ps
                        nc.tensor.matmul(wt_ps[:], zeros_S[:], zeros_S[:, 0:BAG],
                                         start=True, stop=False)
                    hps = hps_p.tile([TILE, WID], fp32)
                    for j in range(QT):
                        c0 = (jq * QT + j) * 128
                        o = (j * BAG, (j + 1) * BAG)
                        if use_bf16h:
                            nc.tensor.matmul(hps[:, o[0]:o[1]],
                                             xa[:, c0:c0 + 128], wt0h[:],
                                             start=True, stop=False)
                            nc.tensor.matmul(hps[:, o[0]:o[1]],
                                             xa[:, c0:c0 + 128], wt0l[:],
                                             start=False, stop=False)
                            nc.tensor.matmul(hps[:, o[0]:o[1]],
                                             xb[:, c0:c0 + 128], wt1h[:],
                                             start=False, stop=False)
                            nc.tensor.matmul(hps[:, o[0]:o[1]],
                                             xb[:, c0:c0 + 128], wt1l[:],
                                             start=False, stop=False)
                            nc.tensor.matmul(hps[:, o[0]:o[1]],
                                             xa[:, gi + c0:gi + c0 + 128], wt0h[:],
                                             start=False, stop=False)
                            nc.tensor.matmul(hps[:, o[0]:o[1]],
                                             xb[:, gi + c0:gi + c0 + 128], wt1h[:],
                                             start=False, stop=True)
                        else:
                            nc.tensor.matmul(hps[:, o[0]:o[1]],
                                             xa[:, c0:c0 + 128], wt0[:],
                                             start=True, stop=False)
                            nc.tensor.matmul(hps[:, o[0]:o[1]],
                                             xb[:, c0:c0 + 128], wt1[:],
                                             start=False, stop=True)
                    if use_bf16seg:
                        hsb = hsb_p.tile([TILE, WID], bf16, tag="hsb_hi")
                        hlo = hsb_p.tile([TILE, WID], bf16, tag="hsb_lo")
                        nc.scalar.activation(hsb[:], hps[:], AFT.Relu)
                        nc.vector.scalar_tensor_tensor(
                            hlo[:], hps[:], 0.0, hsb[:], AOT.max, AOT.subtract)
                    else:
                        hlo = None
                        hsb = hsb_p.tile([TILE, WID], mmdt)
                    if use_bf16seg:
                        pass
                    elif has_bias:
                        bias4 = bass_mod.AP(
                            tensor=bias_bc.tensor, offset=bias_bc.offset,
                            ap=[bias_bc.ap[0], [0, QT], bias_bc.ap[1]])
                        nc.vector.tensor_tensor(
                            hsb[:].rearrange("p (a b) -> p a b", a=QT),
                            hps[:].rearrange("p (a b) -> p a b", a=QT),
                            bias4, AOT.add)
                        nc.vector.tensor_scalar_max(hsb[:], hsb[:], 0.0)
                    else:
                        if q % 2 == 0:
                            nc.vector.tensor_scalar_max(hsb[:], hps[:], 0.0)
                        else:
                            nc.scalar.activation(hsb[:], hps[:], AFT.Relu)
                    # wide selector: S[p, a*128 + f] = (iota[f] == slots[p, t0q+a])
                    S = s_p.tile([TILE, WID], segdt)
                    scol = slots_sb[:, q * QT:(q + 1) * QT]
                    srep = bass_mod.AP(tensor=scol.tensor, offset=scol.offset,
                                       ap=[scol.ap[0], scol.ap[1], [0, BAG]])
                    ibase = iota_sb[:]
                    irep = bass_mod.AP(tensor=ibase.tensor, offset=ibase.offset,
                                       ap=[ibase.ap[0], [0, QT], ibase.ap[1]])
                    nc.vector.tensor_tensor(
                        S[:].rearrange("p (a b) -> p a b", a=QT),
                        irep, srep, AOT.is_equal)
                    cur = (q, S, hsb, hlo, w, jq == qpw - 1)
                else:
                    cur = None
                if prev is not None:
                    pq, pS, phsb, phlo, pw, plast = prev
                    for j in range(QT):
                        last = plast and j == QT - 1
                        nc.tensor.matmul(wps_tiles[pw][:],
                                         pS[:, j * BAG:(j + 1) * BAG],
                                         phsb[:, j * BAG:(j + 1) * BAG],
                                         start=False,
                                         stop=(last and phlo is None))
                        if phlo is not None:
                            nc.tensor.matmul(wps_tiles[pw][:],
                                             pS[:, j * BAG:(j + 1) * BAG],
                                             phlo[:, j * BAG:(j + 1) * BAG],
                                             start=False, stop=last)
                    if plast:
                        aggw = agg_big[:, pw * BAG:(pw + 1) * BAG]
                        nc.scalar.activation(aggw, wps_tiles[pw][:], AFT.Copy,
                                             scale=recip_sb[:, pw:pw + 1])
                        a2 = agg2_p.tile([TILE, BAG], fp32)
                        nc.scalar.square(a2[:], aggw)
                        nc.tensor.matmul(stats_a[:], mask_sb[:, pw:pw + 1], aggw,
                                         start=(pw == 0), stop=(pw == ng - 1))
                        nc.tensor.matmul(stats_b[:], mask_sb[:, pw:pw + 1], a2[:],
                                         start=(pw == 0), stop=(pw == ng - 1))
                        del wps_tiles[pw]
                prev = cur

            # ---------------- stats all-reduce + params ----------------
            stats_sb = small_p.tile([1, 2 * BAG], fp32)
            nc.vector.tensor_copy(stats_sb[0:1, 0:BAG], stats_a[:])
            nc.vector.tensor_copy(stats_sb[0:1, BAG:2 * BAG], stats_b[:])
            cc_in = dram_p.tile([1, 2 * BAG], fp32)
            cc_out = dram_p.tile([1, 2 * BAG], fp32)
            nc.sync.dma_start(cc_in[:], stats_sb[:])
            nc.gpsimd.collective_compute(
                "AllReduce", AOT.add,
                replica_groups=[list(range(N_CORES))],
                ins=[cc_in.opt()], outs=[cc_out.opt()])
            gstats = small_p.tile([1, 2 * BAG], fp32)
            nc.sync.dma_start(gstats[:], cc_out[:])

            inv_nb = 1.0 / float(n_bags)
            mean = small_p.tile([1, BAG], fp32)
            nc.vector.tensor_scalar_mul(mean[:], gstats[0:1, 0:BAG], inv_nb)
            ex2 = small_p.tile([1, BAG], fp32)
            nc.vector.tensor_scalar_mul(ex2[:], gstats[0:1, BAG:2 * BAG], inv_nb)
            m2 = small_p.tile([1, BAG], fp32)
            nc.vector.tensor_tensor(m2[:], mean[:], mean[:], AOT.mult)
            vareps = small_p.tile([1, BAG], fp32)
            nc.vector.tensor_tensor(vareps[:], ex2[:], m2[:], AOT.subtract)
            nc.vector.tensor_scalar_add(vareps[:], vareps[:], EPS)
            rec = small_p.tile([1, BAG], fp32)
            nc.vector.reciprocal(rec[:], vareps[:])
            inv = small_p.tile([1, BAG], fp32)
            nc.scalar.sqrt(inv[:], rec[:])
            ab_row = small_p.tile([1, 2 * BAG], fp32)
            nc.vector.tensor_tensor(ab_row[0:1, 0:BAG], inv[:], grow[:], AOT.mult)
            mA = small_p.tile([1, BAG], fp32)
            nc.vector.tensor_tensor(mA[:], mean[:], ab_row[0:1, 0:BAG], AOT.mult)
            nc.vector.tensor_tensor(ab_row[0:1, BAG:2 * BAG], brow[:], mA[:],
                                    AOT.subtract)
            ab_ps = ab_p.tile([TILE, 2 * BAG], fp32)
            nc.tensor.matmul(ab_ps[:], ones_row[:], ab_row[:], start=True, stop=True)
            ab_sb = constp.tile([TILE, 2 * BAG], fp32)
            nc.vector.tensor_copy(ab_sb[:], ab_ps[:])

            # ---------------- phase 2: normalize + store ----------------
            a_col = ab_sb[:, 0:BAG]
            b_col = ab_sb[:, BAG:2 * BAG]
            w2 = 0
            while w2 < ng:
                nw = min(4, ng - w2)
                wid2 = nw * BAG
                arep = bass_mod.AP(tensor=a_col.tensor, offset=a_col.offset,
                                   ap=[a_col.ap[0], [0, nw], a_col.ap[1]])
                brep = bass_mod.AP(tensor=b_col.tensor, offset=b_col.offset,
                                   ap=[b_col.ap[0], [0, nw], b_col.ap[1]])
                ot = out_p.tile([TILE, 4 * BAG], fp32)
                src = agg_big[:, w2 * BAG:(w2 + nw) * BAG]
                nc.vector.tensor_tensor(
                    ot[:, 0:wid2].rearrange("p (a b) -> p a b", a=nw),
                    src.rearrange("p (a b) -> p a b", a=nw), arep, AOT.mult)
                nc.vector.tensor_tensor(
                    ot[:, 0:wid2].rearrange("p (a b) -> p a b", a=nw),
                    ot[:, 0:wid2].rearrange("p (a b) -> p a b", a=nw),
                    brep, AOT.add)
                # out rows for nw windows are contiguous: [w2*128, (w2+nw)*128)
                nc.sync.dma_start(
                    out[w2 * TILE:(w2 + nw) * TILE, :].rearrange(
                        "(a p) b -> p a b", p=TILE),
                    ot[:, 0:wid2].rearrange("p (a b) -> p a b", a=nw))
                w2 += nw

    nc.compile()
    return nc


# ----------------------------------------------------------------------------
# Entry point
# ----------------------------------------------------------------------------

def kernel(**inputs):
    global LAST_RESULTS
    from concourse.bass_utils import run_bass_kernel_spmd

    x = np.asarray(inputs["x"], dtype=np.float32)
    W = np.asarray(inputs["W"], dtype=np.float32)
    b = np.asarray(inputs["b"], dtype=np.float32)
    gamma = np.asarray(inputs["gamma"], dtype=np.float32)
    beta = np.asarray(inputs["beta"], dtype=np.float32)
    seg_ids = np.asarray(inputs["seg_ids"]).astype(np.int64)
    bags_len = np.asarray(inputs["bags_len"]).astype(np.int64)

    import os
    bf_mode = int(os.environ.get("KERNEL_BF16H", "4"))
    has_bias = bool(np.any(b != 0))
    if bf_mode >= 4 and has_bias:
        bf_mode = 3

    if bf_mode >= 4:
        plan, ng, in_maps, n_bags = _host_prep_t(
            x, W, gamma, beta, seg_ids, bags_len)
        key = (4, ng, n_bags)
        if key not in _NC_CACHE:
            _NC_CACHE[key] = _build_nc_t(ng, n_bags)
        nc = _NC_CACHE[key]
        res = run_bass_kernel_spmd(nc, in_maps, core_ids=list(range(N_CORES)))
        LAST_RESULTS = res
        out_full = np.zeros((n_bags, BAG), dtype=np.float32)
        for c in range(N_CORES):
            oc = np.asarray(res.results[c]["outT"]).astype(np.float32)
            for g, (p, e, fb, lbx) in enumerate(plan[c]["groups"]):
                ns = lbx - fb
                out_full[fb:lbx] = oc[:, g * SLOT_T: g * SLOT_T + ns].T
        return out_full

    plan, t0, ng, in_maps, n_bags = _host_prep(
        x, W, b, gamma, beta, seg_ids, bags_len, bf_mode)

    use_f32r = os.environ.get("KERNEL_F32R", "0") == "1"
    use_bf16h = 1 <= bf_mode <= 2
    use_bf16seg = bf_mode == 2 and not has_bias
    key = (ng, t0, n_bags, has_bias, use_f32r, bf_mode)
    if key not in _NC_CACHE:
        if bf_mode >= 3:
            _NC_CACHE[key] = _build_nc_pure(ng, t0, n_bags, has_bias)
        else:
            _NC_CACHE[key] = _build_nc(ng, t0, n_bags, has_bias,
                                       use_f32r=use_f32r, use_bf16h=use_bf16h,
                                       use_bf16seg=use_bf16seg)
    nc = _NC_CACHE[key]

    res = run_bass_kernel_spmd(nc, in_maps, core_ids=list(range(N_CORES)))
    LAST_RESULTS = res

    out_full = np.zeros((n_bags, BAG), dtype=np.float32)
    for c in range(N_CORES):
        oc = res.results[c]["out"]
        for g, (p, e, fb, lbx) in enumerate(plan[c]["groups"]):
            ns = lbx - fb
            out_full[fb:lbx] = oc[g * TILE: g * TILE + ns]
    return out_full


# revision 28
# speedup vs baseline: 1.1298x; 1.0153x over previous
"""Trainium2 Bass kernel for nn_BagInput (segment_reduce + linear/relu + BatchNorm).

Computation (matches the reference):
    h   = relu(x @ W.T + b)                      [N_items, 128]
    agg = segment_mean(h, seg_ids, NB)           [NB, 128]   (empty bags -> 0)
    out = (agg - mean) * rsqrt(var + eps) * gamma + beta   (batch stats over bags)

Strategy (8 NeuronCores, data-parallel over items, bag-aligned shards):
  - Host: shard items at bag boundaries; per core, pack items into groups of
    T0=16 128-item tiles, padding each group so that group boundaries fall on
    bag boundaries (~0.5% padding). One group == one "window" of <=128 bags.
  - Host: pre-transpose x to feature-major (xT) so the per-tile [feat, item]
    blocks DMA directly as matmul stationary operands. x is cast to bf16
    (mode 3, default): the rel-err budget (2e-2) dwarfs bf16 rounding
    (~0.5%), and it halves HBM traffic vs hi/lo-split f32 emulation.
  - Device per 128-item tile:
      h_psum = xT0_t.T @ WT0 + xT1_t.T @ WT1      (PE, K=256 in 2 chunks)
      h_sbuf = relu(h_psum) -> bf16               (DVE / ACT alternating)
      S      = (iota_row == slot_ids_t)           (0/1 selector, DVE, bf16)
      wps_w += S.T @ h_sbuf                       (PE; window accumulate in PSUM)
  - Window drain: agg = wps * (1/cnt)  (per-partition scalar); accumulate BN
    partial stats via mask.T @ [agg, agg^2] matmuls into PSUM.
  - AllReduce (8 cores) of the [1,256] stats; compute A = gamma*rsqrt(var+eps),
    B = beta - mean*A; broadcast via ones-matmul; out = agg*A + B; DMA out.
  - Host: gather per-(window,slot) rows back to global bag order.
"""

import numpy as np

N_CORES = 8
TILE = 128
FEAT = 256
BAG = 128
EPS = 1e-5

_NC_CACHE = {}
LAST_RESULTS = None  # BassKernelResults of the most recent run (for profiling)


# ----------------------------------------------------------------------------
# Host-side planning
# ----------------------------------------------------------------------------

def _plan_cores(seg_ids, n_bags, t0):
    """Split items/bags across cores at bag boundaries; pack groups of t0
    tiles per core such that each group covers whole bags (<=128 bags)."""
    gi = t0 * TILE
    n = seg_ids.shape[0]
    cuts = [0]
    bag_cuts = [0]
    for c in range(1, N_CORES):
        tgt = (n * c) // N_CORES
        bb = int(seg_ids[tgt])
        cut = int(np.searchsorted(seg_ids, bb, side="left"))
        cuts.append(cut)
        bag_cuts.append(bb)
    cuts.append(n)
    bag_cuts.append(n_bags)
    for c in range(N_CORES):
        if cuts[c + 1] <= cuts[c]:
            raise ValueError("degenerate core split")

    cores = []
    for c in range(N_CORES):
        i0, i1 = cuts[c], cuts[c + 1]
        b0, b1 = bag_cuts[c], bag_cuts[c + 1]
        seg = seg_ids[i0:i1]
        nloc = i1 - i0
        groups = []
        p = 0
        fb = b0
        while p < nloc:
            if p + gi >= nloc:
                e = nloc
                lbx = b1
            else:
                e = int(np.searchsorted(seg, seg[p + gi], side="left"))
                if e <= p:
                    raise ValueError("single bag larger than group size")
                lbx = int(seg[e - 1]) + 1
            if lbx - fb > TILE:
                raise ValueError(f"window spans {lbx - fb} bags > {TILE}")
            groups.append((p, e, fb, lbx))
            fb = lbx
            p = e
        cores.append(dict(i0=i0, i1=i1, b0=b0, b1=b1, groups=groups))
    return cores


def _host_prep(x, W, b, gamma, beta, seg_ids, bags_len, bf_mode):
    n_bags = bags_len.shape[0]
    plan = None
    for t0 in (16, 8, 4):
        try:
            plan = _plan_cores(seg_ids, n_bags, t0)
            break
        except ValueError:
            continue
    if plan is None:
        raise ValueError("could not plan groups")
    gi = t0 * TILE

    ng = max(len(c["groups"]) for c in plan)
    nt = ng * t0
    npad = ng * gi

    cnt = np.maximum(bags_len, 1).astype(np.float32)
    recip_all = 1.0 / cnt

    import ml_dtypes
    bf = ml_dtypes.bfloat16

    in_maps = []
    for c in range(N_CORES):
        info = plan[c]
        i0 = info["i0"]
        seg = seg_ids[i0:info["i1"]]
        groups = info["groups"]

        idx = np.full(npad, -1, dtype=np.int64)
        slots = np.full(npad, 255.0, dtype=np.float32)
        recip = np.ones((ng, TILE), dtype=np.float32)
        mask = np.zeros((ng, TILE), dtype=np.float32)
        for g, (p, e, fb, lbx) in enumerate(groups):
            m = e - p
            idx[g * gi: g * gi + m] = i0 + p + np.arange(m)
            slots[g * gi: g * gi + m] = (seg[p:e] - fb).astype(np.float32)
            ns = lbx - fb
            recip[g, :ns] = recip_all[fb:lbx]
            mask[g, :ns] = 1.0

        xp = np.zeros((npad, FEAT), dtype=np.float32)
        valid = idx >= 0
        xp[valid] = x[idx[valid]]
        xT = np.ascontiguousarray(xp.T)
        del xp

        common = {
            "recip": np.ascontiguousarray(recip.T),
            "mask": np.ascontiguousarray(mask.T),
            "gamma_row": np.ascontiguousarray(gamma.reshape(1, BAG)),
            "beta_row": np.ascontiguousarray(beta.reshape(1, BAG)),
            "bias_bc": np.ascontiguousarray(
                np.tile(b.reshape(1, BAG), (TILE, 1))),
        }
        if bf_mode >= 3:
            # pure bf16: single-precision x and W, halve HBM traffic
            in_maps.append({
                "xTh": np.ascontiguousarray(xT.astype(bf)),
                "WTh": np.ascontiguousarray(W.T.astype(bf)),
                "slots": np.ascontiguousarray(slots.reshape(nt, TILE).T.astype(bf)),
                "iota": np.ascontiguousarray(
                    np.tile(np.arange(TILE, dtype=np.float32), (TILE, 1)).astype(bf)),
                **common,
            })
        elif bf_mode >= 1:
            xTh = xT.astype(bf)
            xTl = (xT - xTh.astype(np.float32)).astype(bf)
            WTf = np.ascontiguousarray(W.T)
            WTh = WTf.astype(bf)
            WTl = (WTf - WTh.astype(np.float32)).astype(bf)
            in_maps.append({
                "xTh": np.ascontiguousarray(xTh),
                "xTl": np.ascontiguousarray(xTl),
                "WTh": np.ascontiguousarray(WTh),
                "WTl": np.ascontiguousarray(WTl),
                "slots": np.ascontiguousarray(slots.reshape(nt, TILE).T.astype(bf)),
                "iota": np.ascontiguousarray(
                    np.tile(np.arange(TILE, dtype=np.float32), (TILE, 1)).astype(bf)),
                **common,
            })
        else:
            in_maps.append({
                "xT": xT,
                "slots": np.ascontiguousarray(slots.reshape(nt, TILE).T),
                "WT": np.ascontiguousarray(W.T),
                "iota": np.ascontiguousarray(
                    np.tile(np.arange(TILE, dtype=np.float32), (TILE, 1))),
                **common,
            })
    return plan, t0, ng, in_maps, n_bags


# ----------------------------------------------------------------------------
# Mode 4: transposed seg-matmul, 64-slot windows, recip folded into x
# ----------------------------------------------------------------------------

T0_T = 8              # tiles per window (1024 items)
SLOT_T = 64           # one-hot width (<=64 bags per window)


def _plan_cores_t(seg_ids, n_bags):
    """Core split at bag boundaries; per core, windows of <=T0_T*128 items
    AND <=SLOT_T bags, each window covering whole bags."""
    gi = T0_T * TILE
    n = seg_ids.shape[0]
    cuts = [0]
    bag_cuts = [0]
    for c in range(1, N_CORES):
        tgt = (n * c) // N_CORES
        bb = int(seg_ids[tgt])
        cut = int(np.searchsorted(seg_ids, bb, side="left"))
        cuts.append(cut)
        bag_cuts.append(bb)
    cuts.append(n)
    bag_cuts.append(n_bags)
    for c in range(N_CORES):
        if cuts[c + 1] <= cuts[c]:
            raise ValueError("degenerate core split")

    cores = []
    for c in range(N_CORES):
        i0, i1 = cuts[c], cuts[c + 1]
        b0, b1 = bag_cuts[c], bag_cuts[c + 1]
        seg = seg_ids[i0:i1]
        nloc = i1 - i0
        groups = []
        p = 0
        fb = b0
        while p < nloc:
            if p + gi >= nloc:
                e = nloc
                lbx = b1
            else:
                e = int(np.searchsorted(seg, seg[p + gi], side="left"))
                if e <= p:
                    raise ValueError("single bag larger than window")
                lbx = int(seg[e - 1]) + 1
            if lbx - fb > SLOT_T:
                lbx = fb + SLOT_T
                e = int(np.searchsorted(seg, lbx, side="left"))
                if e <= p:
                    raise ValueError("bag-capped window is empty")
            groups.append((p, e, fb, lbx))
            fb = lbx
            p = e
        cores.append(dict(i0=i0, i1=i1, b0=b0, b1=b1, groups=groups))
    return cores


def _host_prep_t(x, W, gamma, beta, seg_ids, bags_len):
    n_bags = bags_len.shape[0]
    plan = _plan_cores_t(seg_ids, n_bags)
    gi = T0_T * TILE

    ng = max(len(c["groups"]) for c in plan)
    if ng % 2:
        ng += 1
    nt = ng * T0_T
    npad = ng * gi

    cnt = np.maximum(bags_len, 1).astype(np.float32)
    recip_all = (1.0 / cnt).astype(np.float32)
    xs = x * recip_all[np.asarray(seg_ids)][:, None]

    import ml_dtypes
    bf = ml_dtypes.bfloat16

    WT_bf = np.ascontiguousarray(W.T.astype(bf))
    iota64 = np.ascontiguousarray(
        np.tile(np.arange(SLOT_T, dtype=np.float32), (TILE, 1)).astype(bf))

    in_maps = []
    for c in range(N_CORES):
        info = plan[c]
        i0 = info["i0"]
        seg = seg_ids[i0:info["i1"]]
        groups = info["groups"]

        idx = np.full(npad, -1, dtype=np.int64)
        slots = np.full(npad, 255.0, dtype=np.float32)
        for g, (p, e, fb, lbx) in enumerate(groups):
            m = e - p
            idx[g * gi: g * gi + m] = i0 + p + np.arange(m)
            slots[g * gi: g * gi + m] = (seg[p:e] - fb).astype(np.float32)

        xp = np.zeros((npad, FEAT), dtype=np.float32)
        valid = idx >= 0
        xp[valid] = xs[idx[valid]]
        xT = np.ascontiguousarray(xp.T.astype(bf))
        del xp

        in_maps.append({
            "xT": xT,
            "WT": WT_bf,
            "slots": np.ascontiguousarray(slots.reshape(nt, TILE).T.astype(bf)),
            "iota64": iota64,
            "gamma_col": np.ascontiguousarray(gamma.reshape(BAG, 1)),
            "beta_col": np.ascontiguousarray(beta.reshape(BAG, 1)),
        })
    return plan, ng, in_maps, n_bags


def _build_nc_t(ng, n_bags):
    import os
    use_ttr = os.environ.get("KERNEL_TTR", "0") == "1"
    use_cc = os.environ.get("KERNEL_NOCC", "0") != "1"
    import concourse.bacc as bacc
    import concourse.tile as tile
    import concourse.mybir as mybir
    import concourse.bass as bass_mod

    fp32 = mybir.dt.float32
    bf16 = mybir.dt.bfloat16
    AOT = mybir.AluOpType
    AFT = mybir.ActivationFunctionType

    gi = T0_T * TILE
    nt = ng * T0_T
    npad = ng * gi
    np2 = ng // 2
    SW = T0_T * SLOT_T          # selector cols per window (512)

    nc = bacc.Bacc("TRN2", target_bir_lowering=False, debug=False,
                   enable_asserts=False, num_devices=N_CORES)
    xT = nc.dram_tensor("xT", [FEAT, npad], bf16, kind="ExternalInput")
    WT_in = nc.dram_tensor("WT", [FEAT, BAG], bf16, kind="ExternalInput")
    slots = nc.dram_tensor("slots", [TILE, nt], bf16, kind="ExternalInput")
    iota_in = nc.dram_tensor("iota64", [TILE, SLOT_T], bf16, kind="ExternalInput")
    gcol_in = nc.dram_tensor("gamma_col", [BAG, 1], fp32, kind="ExternalInput")
    bcol_in = nc.dram_tensor("beta_col", [BAG, 1], fp32, kind="ExternalInput")
    outT = nc.dram_tensor("outT", [TILE, np2 * TILE], bf16, kind="ExternalOutput")

    with tile.TileContext(nc) as tc:
        with tc.tile_pool(name="const", bufs=1) as constp, \
             tc.tile_pool(name="xa", bufs=4) as xa_p, \
             tc.tile_pool(name="xb", bufs=4) as xb_p, \
             tc.tile_pool(name="hsb", bufs=4) as hsb_p, \
             tc.tile_pool(name="Sp", bufs=4) as s_p, \
             tc.tile_pool(name="agg", bufs=1) as agg_p, \
             tc.tile_pool(name="sq", bufs=2) as sq_p, \
             tc.tile_pool(name="stat", bufs=4) as stat_p, \
             tc.tile_pool(name="outp", bufs=2) as out_p, \
             tc.tile_pool(name="small", bufs=1) as small_p, \
             tc.tile_pool(name="hps", bufs=3, space="PSUM") as hps_p, \
             tc.tile_pool(name="wpsp", bufs=2, space="PSUM") as wps_p, \
             tc.tile_pool(name="dram", bufs=1, space="DRAM") as dram_p:

            wt0 = constp.tile([128, BAG], bf16)
            nc.sync.dma_start(wt0[:], WT_in[0:128, :])
            wt1 = constp.tile([128, BAG], bf16)
            nc.sync.dma_start(wt1[:], WT_in[128:256, :])
            iota_sb = constp.tile([TILE, SLOT_T], bf16)
            nc.sync.dma_start(iota_sb[:], iota_in[:, :])
            slots_sb = constp.tile([TILE, nt], bf16)
            nc.sync.dma_start(slots_sb[:], slots[:, :])
            gcol = constp.tile([BAG, 1], fp32)
            nc.sync.dma_start(gcol[:], gcol_in[:, :])
            bcol = constp.tile([BAG, 1], fp32)
            nc.sync.dma_start(bcol[:], bcol_in[:, :])
            zcol = constp.tile([TILE, 1], fp32)
            nc.vector.memset(zcol[:], 0.0)

            agg_big = agg_p.tile([TILE, np2 * BAG], fp32)
            if not use_ttr:
                sa_tab = constp.tile([TILE, np2], fp32)
                sb_tab = constp.tile([TILE, np2], fp32)

            wps_tiles = {}
            prev = None
            sa_prev = 0.0
            sb_prev = 0.0
            for q in range(ng + 1):
                if q < ng:
                    w = q
                    if w % 2 == 0:
                        xa = xa_p.tile([128, 2 * gi], bf16, tag="xa")
                        nc.sync.dma_start(xa[:], xT[0:128, w * gi:(w + 2) * gi])
                        xb = xb_p.tile([128, 2 * gi], bf16, tag="xb")
                        nc.sync.dma_start(xb[:], xT[128:256, w * gi:(w + 2) * gi])
                    x0 = (w % 2) * gi
                    if w % 2 == 0:
                        # full-bank tile so the pair's accumulation groups and
                        # the drain never share a PSUM bank with another pair
                        wt_full = wps_p.tile([TILE, 512], fp32)
                        wps_tiles[w // 2] = wt_full
                    HW2 = T0_T * BAG // 2            # 512 = one PSUM bank
                    hpsa = hps_p.tile([TILE, HW2], fp32, tag="hpsa")
                    hpsb = hps_p.tile([TILE, HW2], fp32, tag="hpsb")
                    for j in range(T0_T):
                        hp = hpsa if j < 4 else hpsb
                        o = ((j % 4) * BAG, (j % 4 + 1) * BAG)
                        c0 = x0 + j * 128
                        nc.tensor.matmul(hp[:, o[0]:o[1]],
                                         xa[:, c0:c0 + 128], wt0[:],
                                         start=True, stop=False)
                        nc.tensor.matmul(hp[:, o[0]:o[1]],
                                         xb[:, c0:c0 + 128], wt1[:],
                                         start=False, stop=True)
                    hsb = hsb_p.tile([TILE, T0_T * BAG], bf16)
                    nc.scalar.activation(hsb[:, 0:HW2], hpsa[:], AFT.Relu)
                    nc.scalar.activation(hsb[:, HW2:2 * HW2], hpsb[:], AFT.Relu)
                    # selector: S[p, a*64 + f] = (iota64[f] == slots[p, 8w+a])
                    S = s_p.tile([TILE, SW], bf16)
                    scol = slots_sb[:, w * T0_T:(w + 1) * T0_T]
                    srep = bass_mod.AP(tensor=scol.tensor, offset=scol.offset,
                                       ap=[scol.ap[0], scol.ap[1], [0, SLOT_T]])
                    ibase = iota_sb[:]
                    irep = bass_mod.AP(tensor=ibase.tensor, offset=ibase.offset,
                                       ap=[ibase.ap[0], [0, T0_T], ibase.ap[1]])
                    nc.vector.tensor_tensor(
                        S[:].rearrange("p (a b) -> p a b", a=T0_T),
                        irep, srep, AOT.is_equal)
                    cur = (w, hsb, S)
                else:
                    cur = None
                if prev is not None:
                    pw, phsb, pS = prev
                    # windows of a pair land at 512B-aligned cols 0 and 128
                    h0 = (pw % 2) * BAG
                    k = pw // 2
                    wt_ps = wps_tiles[k]
                    for j in range(T0_T):
                        nc.tensor.matmul(wt_ps[:, h0:h0 + SLOT_T],
                                         phsb[:, j * BAG:(j + 1) * BAG],
                                         pS[:, j * SLOT_T:(j + 1) * SLOT_T],
                                         start=(j == 0), stop=(j == T0_T - 1))
                    if pw % 2 == 1:
                        aggsl = agg_big[:, k * BAG:(k + 1) * BAG]
                        zrep = bass_mod.AP(tensor=zcol.tensor, offset=zcol.offset,
                                           ap=[zcol.ap[0], [0, 2], [0, SLOT_T]])
                        wsrc = wt_ps[:]
                        wstr = bass_mod.AP(tensor=wsrc.tensor, offset=wsrc.offset,
                                           ap=[wsrc.ap[0], [BAG, 2], [1, SLOT_T]])
                        if use_ttr:
                            sa_new = stat_p.tile([TILE, 1], fp32, tag="sa")
                            nc.vector.tensor_tensor_reduce(
                                aggsl.rearrange("p (a b) -> p a b", a=2),
                                wstr, zrep, 1.0, sa_prev,
                                AOT.add, AOT.add, sa_new[:])
                            sq = sq_p.tile([TILE, BAG], fp32)
                            sb_new = stat_p.tile([TILE, 1], fp32, tag="sb")
                            nc.vector.tensor_tensor_reduce(
                                sq[:], aggsl, aggsl, 1.0, sb_prev,
                                AOT.mult, AOT.add, sb_new[:])
                            sa_prev = sa_new[:]
                            sb_prev = sb_new[:]
                        else:
                            nc.vector.scalar_tensor_tensor(
                                aggsl.rearrange("p (a b) -> p a b", a=2),
                                wstr, 1.0, zrep, AOT.mult, AOT.add,
                                accum_out=sa_tab[:, k:k + 1])
                            sq = sq_p.tile([TILE, BAG], fp32)
                            nc.vector.scalar_tensor_tensor(
                                sq[:], aggsl, 0.0, aggsl, AOT.add, AOT.mult,
                                accum_out=sb_tab[:, k:k + 1])
                        del wps_tiles[k]
                prev = cur

            # ---------------- stats all-reduce + params ----------------
            stats_sb = small_p.tile([TILE, 2], fp32)
            if use_ttr:
                nc.vector.tensor_copy(stats_sb[:, 0:1], sa_prev)
                nc.vector.tensor_copy(stats_sb[:, 1:2], sb_prev)
            else:
                nc.vector.tensor_reduce(stats_sb[:, 0:1], sa_tab[:],
                                        mybir.AxisListType.X, AOT.add)
                nc.vector.tensor_reduce(stats_sb[:, 1:2], sb_tab[:],
                                        mybir.AxisListType.X, AOT.add)
            gstats = small_p.tile([TILE, 2], fp32)
            if use_cc:
                # flat [1, 256] views so the CC reduces one contiguous row
                cc_in = dram_p.tile([1, 2 * BAG], fp32)
                cc_out = dram_p.tile([1, 2 * BAG], fp32)
                cc_in_t = cc_in[0:1, :].rearrange("a (p b) -> p a b", p=TILE)
                cc_out_t = cc_out[0:1, :].rearrange("a (p b) -> p a b", p=TILE)
                nc.sync.dma_start(
                    cc_in_t, stats_sb[:].rearrange("p (a b) -> p a b", a=1))
                nc.gpsimd.collective_compute(
                    "AllReduce", AOT.add,
                    replica_groups=[list(range(N_CORES))],
                    ins=[cc_in.opt()], outs=[cc_out.opt()])
                nc.sync.dma_start(
                    gstats[:].rearrange("p (a b) -> p a b", a=1), cc_out_t)
            else:
                nc.vector.tensor_copy(gstats[:], stats_sb[:])

            inv_nb = 1.0 / float(n_bags)
            mean = small_p.tile([TILE, 1], fp32)
            nc.vector.tensor_scalar_mul(mean[:], gstats[:, 0:1], inv_nb)
            ex2 = small_p.tile([TILE, 1], fp32)
            nc.vector.tensor_scalar_mul(ex2[:], gstats[:, 1:2], inv_nb)
            m2 = small_p.tile([TILE, 1], fp32)
            nc.vector.tensor_tensor(m2[:], mean[:], mean[:], AOT.mult)
            vareps = small_p.tile([TILE, 1], fp32)
            nc.vector.tensor_tensor(vareps[:], ex2[:], m2[:], AOT.subtract)
            nc.vector.tensor_scalar_add(vareps[:], vareps[:], EPS)
            rec = small_p.tile([TILE, 1], fp32)
            nc.vector.reciprocal(rec[:], vareps[:])
            inv = small_p.tile([TILE, 1], fp32)
            nc.scalar.sqrt(inv[:], rec[:])
            acol = small_p.tile([TILE, 1], fp32)
            nc.vector.tensor_tensor(acol[:], inv[:], gcol[:], AOT.mult)
            mA = small_p.tile([TILE, 1], fp32)
            nc.vector.tensor_tensor(mA[:], mean[:], acol[:], AOT.mult)
            bcol2 = small_p.tile([TILE, 1], fp32)
            nc.vector.tensor_tensor(bcol2[:], bcol[:], mA[:], AOT.subtract)

            # ---------------- phase 2: normalize + store ----------------
            CH = 512
            tot = np2 * BAG
            w2 = 0
            while w2 < tot:
                cw = min(CH, tot - w2)
                brep = bass_mod.AP(tensor=bcol2.tensor, offset=bcol2.offset,
                                   ap=[bcol2.ap[0], [0, cw]])
                ot = out_p.tile([TILE, CH], bf16)
                nc.vector.scalar_tensor_tensor(
                    ot[:, 0:cw], agg_big[:, w2:w2 + cw], acol[:], brep,
                    AOT.mult, AOT.add)
                nc.sync.dma_start(outT[:, w2:w2 + cw], ot[:, 0:cw])
                w2 += cw

    nc.compile()
    return nc


# ----------------------------------------------------------------------------
# Device kernel (pure bf16, mode 3)
# ----------------------------------------------------------------------------

def _build_nc_pure(ng, t0, n_bags, has_bias):
    import concourse.bacc as bacc
    import concourse.tile as tile
    import concourse.mybir as mybir
    import concourse.bass as bass_mod

    fp32 = mybir.dt.float32
    bf16 = mybir.dt.bfloat16
    AOT = mybir.AluOpType
    AFT = mybir.ActivationFunctionType

    gi = t0 * TILE
    nt = ng * t0

    npad = ng * gi

    nc = bacc.Bacc("TRN2", target_bir_lowering=False, debug=False,
                   enable_asserts=False, num_devices=N_CORES)
    xTh = nc.dram_tensor("xTh", [FEAT, npad], bf16, kind="ExternalInput")
    WTh_in = nc.dram_tensor("WTh", [FEAT, BAG], bf16, kind="ExternalInput")
    slots = nc.dram_tensor("slots", [TILE, nt], bf16, kind="ExternalInput")
    recip = nc.dram_tensor("recip", [TILE, ng], fp32, kind="ExternalInput")
    mask = nc.dram_tensor("mask", [TILE, ng], fp32, kind="ExternalInput")
    iota_in = nc.dram_tensor("iota", [TILE, TILE], bf16, kind="ExternalInput")
    grow_in = nc.dram_tensor("gamma_row", [1, BAG], fp32, kind="ExternalInput")
    brow_in = nc.dram_tensor("beta_row", [1, BAG], fp32, kind="ExternalInput")
    bb_in = nc.dram_tensor("bias_bc", [TILE, BAG], fp32, kind="ExternalInput")
    out = nc.dram_tensor("out", [ng * TILE, BAG], fp32, kind="ExternalOutput")

    with tile.TileContext(nc) as tc:
        with tc.tile_pool(name="const", bufs=1) as constp, \
             tc.tile_pool(name="xa", bufs=5) as xa_p, \
             tc.tile_pool(name="xb", bufs=5) as xb_p, \
             tc.tile_pool(name="hsb", bufs=6) as hsb_p, \
             tc.tile_pool(name="Sp", bufs=6) as s_p, \
             tc.tile_pool(name="agg", bufs=1) as agg_p, \
             tc.tile_pool(name="agg2", bufs=2) as agg2_p, \
             tc.tile_pool(name="outp", bufs=2) as out_p, \
             tc.tile_pool(name="small", bufs=1) as small_p, \
             tc.tile_pool(name="hps", bufs=3, space="PSUM") as hps_p, \
             tc.tile_pool(name="wpsp", bufs=2, space="PSUM") as wps_p, \
             tc.tile_pool(name="spsa", bufs=1, space="PSUM") as sps_a_p, \
             tc.tile_pool(name="spsb", bufs=1, space="PSUM") as sps_b_p, \
             tc.tile_pool(name="abps", bufs=1, space="PSUM") as ab_p, \
             tc.tile_pool(name="dram", bufs=1, space="DRAM") as dram_p:

            wt0 = constp.tile([128, BAG], bf16)
            nc.sync.dma_start(wt0[:], WTh_in[0:128, :])
            wt1 = constp.tile([128, BAG], bf16)
            nc.sync.dma_start(wt1[:], WTh_in[128:256, :])
            iota_sb = constp.tile([TILE, TILE], bf16)
            nc.sync.dma_start(iota_sb[:], iota_in[:, :])
            recip_sb = constp.tile([TILE, ng], fp32)
            nc.sync.dma_start(recip_sb[:], recip[:, :])
            mask_sb = constp.tile([TILE, ng], fp32)
            nc.sync.dma_start(mask_sb[:], mask[:, :])
            slots_sb = constp.tile([TILE, nt], bf16)
            nc.sync.dma_start(slots_sb[:], slots[:, :])
            ones_row = constp.tile([1, TILE], fp32)
            nc.vector.memset(ones_row[:], 1.0)
            grow = constp.tile([1, BAG], fp32)
            nc.sync.dma_start(grow[:], grow_in[:, :])
            brow = constp.tile([1, BAG], fp32)
            nc.sync.dma_start(brow[:], brow_in[:, :])
            if has_bias:
                bias_bc = constp.tile([TILE, BAG], fp32)
                nc.sync.dma_start(bias_bc[:], bb_in[:, :])

            stats_a = sps_a_p.tile([1, BAG], fp32)
            stats_b = sps_b_p.tile([1, BAG], fp32)
            agg_big = agg_p.tile([TILE, ng * BAG], fp32)

            # ---------------- phase 1: streamed quads (4 tiles each) -------
            QT = 4                  # tiles per quad
            assert t0 % QT == 0
            qpw = t0 // QT          # quads per window
            nq = nt // QT
            WID = QT * BAG          # 512

            wps_tiles = {}
            xa = xb = None
            prev = None
            for q in range(nq + 1):
                if q < nq:
                    w, jq = divmod(q, qpw)
                    if jq == 0:
                        xa = xa_p.tile([128, gi], bf16, tag="xa")
                        nc.sync.dma_start(xa[:], xTh[0:128, w * gi:(w + 1) * gi])
                        xb = xb_p.tile([128, gi], bf16, tag="xb")
                        nc.sync.dma_start(xb[:], xTh[128:256, w * gi:(w + 1) * gi])
                        wt_ps = wps_p.tile([TILE, BAG], fp32)
                        wps_tiles[w] = wt_ps
                    hps = hps_p.tile([TILE, WID], fp32)
                    for j in range(QT):
                        c0 = (jq * QT + j) * 128
                        o = (j * BAG, (j + 1) * BAG)
                        nc.tensor.matmul(hps[:, o[0]:o[1]],
                                         xa[:, c0:c0 + 128], wt0[:],
                                         start=True, stop=False)
                        nc.tensor.matmul(hps[:, o[0]:o[1]],
                                         xb[:, c0:c0 + 128], wt1[:],
                                         start=False, stop=True)
                    hsb = hsb_p.tile([TILE, WID], bf16)
                    if has_bias:
                        bias4 = bass_mod.AP(
                            tensor=bias_bc.tensor, offset=bias_bc.offset,
                            ap=[bias_bc.ap[0], [0, QT], bias_bc.ap[1]])
                        nc.vector.tensor_tensor(
                            hsb[:].rearrange("p (a b) -> p a b", a=QT),
                            hps[:].rearrange("p (a b) -> p a b", a=QT),
                            bias4, AOT.add)
                        nc.vector.tensor_scalar_max(hsb[:], hsb[:], 0.0)
                    else:
                        nc.scalar.activation(hsb[:], hps[:], AFT.Relu)
                    # wide selector: S[p, a*128 + f] = (iota[f] == slots[p, t0q+a])
                    S = s_p.tile([TILE, WID], bf16)
                    scol = slots_sb[:, q * QT:(q + 1) * QT]
                    srep = bass_mod.AP(tensor=scol.tensor, offset=scol.offset,
                                       ap=[scol.ap[0], scol.ap[1], [0, BAG]])
                    ibase = iota_sb[:]
                    irep = bass_mod.AP(tensor=ibase.tensor, offset=ibase.offset,
                                       ap=[ibase.ap[0], [0, QT], ibase.ap[1]])
                    nc.vector.tensor_tensor(
                        S[:].rearrange("p (a b) -> p a b", a=QT),
                        irep, srep, AOT.is_equal)
                    cur = (q, S, hsb, w, jq == 0, jq == qpw - 1)
                else:
                    cur = None
                if prev is not None:
                    pq, pS, phsb, pw, pfirst, plast = prev
                    for j in range(QT):
                        nc.tensor.matmul(wps_tiles[pw][:],
                                         pS[:, j * BAG:(j + 1) * BAG],
                                         phsb[:, j * BAG:(j + 1) * BAG],
                                         start=(pfirst and j == 0),
                                         stop=(plast and j == QT - 1))
                    if plast:
                        aggw = agg_big[:, pw * BAG:(pw + 1) * BAG]
                        nc.scalar.activation(aggw, wps_tiles[pw][:], AFT.Copy,
                                             scale=recip_sb[:, pw:pw + 1])
                        a2 = agg2_p.tile([TILE, BAG], fp32)
                        nc.vector.tensor_tensor(a2[:], aggw, aggw, AOT.mult)
                        nc.tensor.matmul(stats_a[:], mask_sb[:, pw:pw + 1], aggw,
                                         start=(pw == 0), stop=(pw == ng - 1))
                        nc.tensor.matmul(stats_b[:], mask_sb[:, pw:pw + 1], a2[:],
                                         start=(pw == 0), stop=(pw == ng - 1))
                        del wps_tiles[pw]
                prev = cur

            # ---------------- stats all-reduce + params ----------------
            stats_sb = small_p.tile([1, 2 * BAG], fp32)
            nc.vector.tensor_copy(stats_sb[0:1, 0:BAG], stats_a[:])
            nc.vector.tensor_copy(stats_sb[0:1, BAG:2 * BAG], stats_b[:])
            cc_in = dram_p.tile([1, 2 * BAG], fp32)
            cc_out = dram_p.tile([1, 2 * BAG], fp32)
            nc.sync.dma_start(cc_in[:], stats_sb[:])
            nc.gpsimd.collective_compute(
                "AllReduce", AOT.add,
                replica_groups=[list(range(N_CORES))],
                ins=[cc_in.opt()], outs=[cc_out.opt()])
            gstats = small_p.tile([1, 2 * BAG], fp32)
            nc.sync.dma_start(gstats[:], cc_out[:])

            inv_nb = 1.0 / float(n_bags)
            mean = small_p.tile([1, BAG], fp32)
            nc.vector.tensor_scalar_mul(mean[:], gstats[0:1, 0:BAG], inv_nb)
            ex2 = small_p.tile([1, BAG], fp32)
            nc.vector.tensor_scalar_mul(ex2[:], gstats[0:1, BAG:2 * BAG], inv_nb)
            m2 = small_p.tile([1, BAG], fp32)
            nc.vector.tensor_tensor(m2[:], mean[:], mean[:], AOT.mult)
            vareps = small_p.tile([1, BAG], fp32)
            nc.vector.tensor_tensor(vareps[:], ex2[:], m2[:], AOT.subtract)
            nc.vector.tensor_scalar_add(vareps[:], vareps[:], EPS)
            rec = small_p.tile([1, BAG], fp32)
            nc.vector.reciprocal(rec[:], vareps[:])
            inv = small_p.tile([1, BAG], fp32)
            nc.scalar.sqrt(inv[:], rec[:])
            ab_row = small_p.tile([1, 2 * BAG], fp32)
            nc.vector.tensor_tensor(ab_row[0:1, 0:BAG], inv[:], grow[:], AOT.mult)
            mA = small_p.tile([1, BAG], fp32)
            nc.vector.tensor_tensor(mA[:], mean[:], ab_row[0:1, 0:BAG], AOT.mult)
            nc.vector.tensor_tensor(ab_row[0:1, BAG:2 * BAG], brow[:], mA[:],
                                    AOT.subtract)
            ab_ps = ab_p.tile([TILE, 2 * BAG], fp32)
            nc.tensor.matmul(ab_ps[:], ones_row[:], ab_row[:], start=True, stop=True)
            ab_sb = constp.tile([TILE, 2 * BAG], fp32)
            nc.vector.tensor_copy(ab_sb[:], ab_ps[:])

            # ---------------- phase 2: normalize + store ----------------
            a_col = ab_sb[:, 0:BAG]
            b_col = ab_sb[:, BAG:2 * BAG]
            w2 = 0
            while w2 < ng:
                nw = min(4, ng - w2)
                wid2 = nw * BAG
                arep = bass_mod.AP(tensor=a_col.tensor, offset=a_col.offset,
                                   ap=[a_col.ap[0], [0, nw], a_col.ap[1]])
                brep = bass_mod.AP(tensor=b_col.tensor, offset=b_col.offset,
                                   ap=[b_col.ap[0], [0, nw], b_col.ap[1]])
                ot = out_p.tile([TILE, 4 * BAG], fp32)
                src = agg_big[:, w2 * BAG:(w2 + nw) * BAG]
                nc.vector.tensor_tensor(
                    ot[:, 0:wid2].rearrange("p (a b) -> p a b", a=nw),
                    src.rearrange("p (a b) -> p a b", a=nw), arep, AOT.mult)
                nc.vector.tensor_tensor(
                    ot[:, 0:wid2].rearrange("p (a b) -> p a b", a=nw),
                    ot[:, 0:wid2].rearrange("p (a b) -> p a b", a=nw),
                    brep, AOT.add)
                # out rows for nw windows are contiguous: [w2*128, (w2+nw)*128)
                nc.sync.dma_start(
                    out[w2 * TILE:(w2 + nw) * TILE, :].rearrange(
                        "(a p) b -> p a b", p=TILE),
                    ot[:, 0:wid2].rearrange("p (a b) -> p a b", a=nw))
                w2 += nw

    nc.compile()
    return nc


# ----------------------------------------------------------------------------
# Device kernel (legacy modes 0-2: f32 / bf16 hi-lo split)
# ----------------------------------------------------------------------------

def _build_nc(ng, t0, n_bags, has_bias, relu_dve_mod=2, sbuild_dve_mod=4,
              use_f32r=False, use_bf16h=False, use_bf16seg=False):
    import concourse.bacc as bacc
    import concourse.tile as tile
    import concourse.mybir as mybir

    fp32 = mybir.dt.float32
    mmdt = mybir.dt.float32r if use_f32r else fp32
    bf16 = mybir.dt.bfloat16
    AOT = mybir.AluOpType
    AFT = mybir.ActivationFunctionType

    gi = t0 * TILE
    nt = ng * t0
    npad = ng * gi

    nc = bacc.Bacc("TRN2", target_bir_lowering=False, debug=False,
                   enable_asserts=False, num_devices=N_CORES)
    if use_bf16h:
        xTh = nc.dram_tensor("xTh", [FEAT, npad], bf16, kind="ExternalInput")
        xTl = nc.dram_tensor("xTl", [FEAT, npad], bf16, kind="ExternalInput")
        WTh_in = nc.dram_tensor("WTh", [FEAT, BAG], bf16, kind="ExternalInput")
        WTl_in = nc.dram_tensor("WTl", [FEAT, BAG], bf16, kind="ExternalInput")
    else:
        xT = nc.dram_tensor("xT", [FEAT, npad], mmdt, kind="ExternalInput")
        WT = nc.dram_tensor("WT", [FEAT, BAG], mmdt, kind="ExternalInput")
    sldt = bf16 if use_bf16h else fp32
    slots = nc.dram_tensor("slots", [TILE, nt], sldt, kind="ExternalInput")
    recip = nc.dram_tensor("recip", [TILE, ng], fp32, kind="ExternalInput")
    mask = nc.dram_tensor("mask", [TILE, ng], fp32, kind="ExternalInput")
    iota_in = nc.dram_tensor("iota", [TILE, TILE], sldt, kind="ExternalInput")
    grow_in = nc.dram_tensor("gamma_row", [1, BAG], fp32, kind="ExternalInput")
    brow_in = nc.dram_tensor("beta_row", [1, BAG], fp32, kind="ExternalInput")
    bb_in = nc.dram_tensor("bias_bc", [TILE, BAG], fp32, kind="ExternalInput")
    out = nc.dram_tensor("out", [ng * TILE, BAG], fp32, kind="ExternalOutput")

    with tile.TileContext(nc) as tc:
        with tc.tile_pool(name="const", bufs=1) as constp, \
             tc.tile_pool(name="xa", bufs=4) as xa_p, \
             tc.tile_pool(name="xb", bufs=4) as xb_p, \
             tc.tile_pool(name="hsb", bufs=6) as hsb_p, \
             tc.tile_pool(name="Sp", bufs=6) as s_p, \
             tc.tile_pool(name="agg", bufs=1) as agg_p, \
             tc.tile_pool(name="agg2", bufs=2) as agg2_p, \
             tc.tile_pool(name="outp", bufs=2) as out_p, \
             tc.tile_pool(name="small", bufs=1) as small_p, \
             tc.tile_pool(name="hps", bufs=3, space="PSUM") as hps_p, \
             tc.tile_pool(name="wpsp", bufs=2, space="PSUM") as wps_p, \
             tc.tile_pool(name="spsa", bufs=1, space="PSUM") as sps_a_p, \
             tc.tile_pool(name="spsb", bufs=1, space="PSUM") as sps_b_p, \
             tc.tile_pool(name="abps", bufs=1, space="PSUM") as ab_p, \
             tc.tile_pool(name="dram", bufs=1, space="DRAM") as dram_p:

            if use_bf16h:
                wt0h = constp.tile([128, BAG], bf16)
                nc.sync.dma_start(wt0h[:], WTh_in[0:128, :])
                wt1h = constp.tile([128, BAG], bf16)
                nc.sync.dma_start(wt1h[:], WTh_in[128:256, :])
                wt0l = constp.tile([128, BAG], bf16)
                nc.sync.dma_start(wt0l[:], WTl_in[0:128, :])
                wt1l = constp.tile([128, BAG], bf16)
                nc.sync.dma_start(wt1l[:], WTl_in[128:256, :])
            else:
                wt0 = constp.tile([128, BAG], mmdt)
                nc.sync.dma_start(wt0[:], WT[0:128, :])
                wt1 = constp.tile([128, BAG], mmdt)
                nc.sync.dma_start(wt1[:], WT[128:256, :])
            iota_sb = constp.tile([TILE, TILE], sldt)
            nc.sync.dma_start(iota_sb[:], iota_in[:, :])
            recip_sb = constp.tile([TILE, ng], fp32)
            nc.sync.dma_start(recip_sb[:], recip[:, :])
            mask_sb = constp.tile([TILE, ng], fp32)
            nc.sync.dma_start(mask_sb[:], mask[:, :])
            slots_sb = constp.tile([TILE, nt], sldt)
            nc.sync.dma_start(slots_sb[:], slots[:, :])
            segdt = bf16 if use_bf16seg else mmdt
            zeros_f32 = constp.tile([TILE, TILE], fp32)
            nc.vector.memset(zeros_f32[:], 0.0)
            if use_f32r or use_bf16seg:
                zeros_S = constp.tile([TILE, TILE], segdt)
                nc.vector.tensor_copy(zeros_S[:], zeros_f32[:])
            else:
                zeros_S = zeros_f32
            ones_row = constp.tile([1, TILE], fp32)
            nc.vector.memset(ones_row[:], 1.0)
            grow = constp.tile([1, BAG], fp32)
            nc.sync.dma_start(grow[:], grow_in[:, :])
            brow = constp.tile([1, BAG], fp32)
            nc.sync.dma_start(brow[:], brow_in[:, :])
            if has_bias:
                bias_bc = constp.tile([TILE, BAG], fp32)
                nc.sync.dma_start(bias_bc[:], bb_in[:, :])

            stats_a = sps_a_p.tile([1, BAG], fp32)
            stats_b = sps_b_p.tile([1, BAG], fp32)
            agg_big = agg_p.tile([TILE, ng * BAG], fp32)

            # ---------------- phase 1: streamed quads (4 tiles each) -------
            QT = 4                  # tiles per quad
            assert t0 % QT == 0
            qpw = t0 // QT          # quads per window
            nq = nt // QT
            WID = QT * BAG          # 512

            import concourse.bass as bass_mod
            wps_tiles = {}
            xa = xb = None
            prev = None
            for q in range(nq + 1):
                if q < nq:
                    w, jq = divmod(q, qpw)
                    if jq == 0:
                        if use_bf16h:
                            xa = xa_p.tile([128, 2 * gi], bf16, tag="xa")
                            nc.sync.dma_start(
                                xa[:, 0:gi], xTh[0:128, w * gi:(w + 1) * gi])
                            nc.sync.dma_start(
                                xa[:, gi:2 * gi], xTl[0:128, w * gi:(w + 1) * gi])
                            xb = xb_p.tile([128, 2 * gi], bf16, tag="xb")
                            nc.sync.dma_start(
                                xb[:, 0:gi], xTh[128:256, w * gi:(w + 1) * gi])
                            nc.sync.dma_start(
                                xb[:, gi:2 * gi], xTl[128:256, w * gi:(w + 1) * gi])
                        else:
                            xa = xa_p.tile([128, gi], mmdt)
                            nc.sync.dma_start(xa[:], xT[0:128, w * gi:(w + 1) * gi])
                            xb = xb_p.tile([128, gi], mmdt)
                            nc.sync.dma_start(xb[:], xT[128:256, w * gi:(w + 1) * gi])
                        wt_ps = wps_p.tile([TILE, BAG], fp32)
                        wps_tiles[w] = wt_ps
                        nc.tensor.matmul(wt_ps[:], zeros_S[:], zeros_S[:, 0:BAG],
                                         start=True, stop=False)
                    hps = hps_p.tile([TILE, WID], fp32)
                    for j in range(QT):
                        c0 = (jq * QT + j) * 128
                        o = (j * BAG, (j + 1) * BAG)
                        if use_bf16h:
                            nc.tensor.matmul(hps[:, o[0]:o[1]],
                                             xa[:, c0:c0 + 128], wt0h[:],
                                             start=True, stop=False)
                            nc.tensor.matmul(hps[:, o[0]:o[1]],
                                             xa[:, c0:c0 + 128], wt0l[:],
                                             start=False, stop=False)
                            nc.tensor.matmul(hps[:, o[0]:o[1]],
                                             xb[:, c0:c0 + 128], wt1h[:],
                                             start=False, stop=False)
                            nc.tensor.matmul(hps[:, o[0]:o[1]],
                                             xb[:, c0:c0 + 128], wt1l[:],
                                             start=False, stop=False)
                            nc.tensor.matmul(hps[:, o[0]:o[1]],
                                             xa[:, gi + c0:gi + c0 + 128], wt0h[:],
                                             start=False, stop=False)
                            nc.tensor.matmul(hps[:, o[0]:o[1]],
                                             xb[:, gi + c0:gi + c0 + 128], wt1h[:],
                                             start=False, stop=True)
                        else:
                            nc.tensor.matmul(hps[:, o[0]:o[1]],
                                             xa[:, c0:c0 + 128], wt0[:],
                                             start=True, stop=False)
                            nc.tensor.matmul(hps[:, o[0]:o[1]],
                                             xb[:, c0:c0 + 128], wt1[:],
                                             start=False, stop=True)
                    if use_bf16seg:
                        hsb = hsb_p.tile([TILE, WID], bf16, tag="hsb_hi")
                        hlo = hsb_p.tile([TILE, WID], bf16, tag="hsb_lo")
                        nc.scalar.activation(hsb[:], hps[:], AFT.Relu)
                        nc.vector.scalar_tensor_tensor(
                            hlo[:], hps[:], 0.0, hsb[:], AOT.max, AOT.subtract)
                    else:
                        hlo = None
                        hsb = hsb_p.tile([TILE, WID], mmdt)
                    if use_bf16seg:
                        pass
                    elif has_bias:
                        bias4 = bass_mod.AP(
                            tensor=bias_bc.tensor, offset=bias_bc.offset,
                            ap=[bias_bc.ap[0], [0, QT], bias_bc.ap[1]])
                        nc.vector.tensor_tensor(
                            hsb[:].rearrange("p (a b) -> p a b", a=QT),
                            hps[:].rearrange("p (a b) -> p a b", a=QT),
                            bias4, AOT.add)
                        nc.vector.tensor_scalar_max(hsb[:], hsb[:], 0.0)
                    else:
                        if q % 2 == 0:
                            nc.vector.tensor_scalar_max(hsb[:], hps[:], 0.0)
                        else:
                            nc.scalar.activation(hsb[:], hps[:], AFT.Relu)
                    # wide selector: S[p, a*128 + f] = (iota[f] == slots[p, t0q+a])
                    S = s_p.tile([TILE, WID], segdt)
                    scol = slots_sb[:, q * QT:(q + 1) * QT]
                    srep = bass_mod.AP(tensor=scol.tensor, offset=scol.offset,
                                       ap=[scol.ap[0], scol.ap[1], [0, BAG]])
                    ibase = iota_sb[:]
                    irep = bass_mod.AP(tensor=ibase.tensor, offset=ibase.offset,
                                       ap=[ibase.ap[0], [0, QT], ibase.ap[1]])
                    nc.vector.tensor_tensor(
                        S[:].rearrange("p (a b) -> p a b", a=QT),
                        irep, srep, AOT.is_equal)
                    cur = (q, S, hsb, hlo, w, jq == qpw - 1)
                else:
                    cur = None
                if prev is not None:
                    pq, pS, phsb, phlo, pw, plast = prev
                    for j in range(QT):
                        last = plast and j == QT - 1
                        nc.tensor.matmul(wps_tiles[pw][:],
                                         pS[:, j * BAG:(j + 1) * BAG],
                                         phsb[:, j * BAG:(j + 1) * BAG],
                                         start=False,
                                         stop=(last and phlo is None))
                        if phlo is not None:
                            nc.tensor.matmul(wps_tiles[pw][:],
                                             pS[:, j * BAG:(j + 1) * BAG],
                                             phlo[:, j * BAG:(j + 1) * BAG],
                                             start=False, stop=last)
                    if plast:
                        aggw = agg_big[:, pw * BAG:(pw + 1) * BAG]
                        nc.scalar.activation(aggw, wps_tiles[pw][:], AFT.Copy,
                                             scale=recip_sb[:, pw:pw + 1])
                        a2 = agg2_p.tile([TILE, BAG], fp32)
                        nc.scalar.square(a2[:], aggw)
                        nc.tensor.matmul(stats_a[:], mask_sb[:, pw:pw + 1], aggw,
                                         start=(pw == 0), stop=(pw == ng - 1))
                        nc.tensor.matmul(stats_b[:], mask_sb[:, pw:pw + 1], a2[:],
                                         start=(pw == 0), stop=(pw == ng - 1))
                        del wps_tiles[pw]
                prev = cur

            # ---------------- stats all-reduce + params ----------------
            stats_sb = small_p.tile([1, 2 * BAG], fp32)
            nc.vector.tensor_copy(stats_sb[0:1, 0:BAG], stats_a[:])
            nc.vector.tensor_copy(stats_sb[0:1, BAG:2 * BAG], stats_b[:])
            cc_in = dram_p.tile([1, 2 * BAG], fp32)
            cc_out = dram_p.tile([1, 2 * BAG], fp32)
            nc.sync.dma_start(cc_in[:], stats_sb[:])
            nc.gpsimd.collective_compute(
                "AllReduce", AOT.add,
                replica_groups=[list(range(N_CORES))],
                ins=[cc_in.opt()], outs=[cc_out.opt()])
            gstats = small_p.tile([1, 2 * BAG], fp32)
            nc.sync.dma_start(gstats[:], cc_out[:])

            inv_nb = 1.0 / float(n_bags)
            mean = small_p.tile([1, BAG], fp32)
            nc.vector.tensor_scalar_mul(mean[:], gstats[0:1, 0:BAG], inv_nb)
            ex2 = small_p.tile([1, BAG], fp32)
            nc.vector.tensor_scalar_mul(ex2[:], gstats[0:1, BAG:2 * BAG], inv_nb)
            m2 = small_p.tile([1, BAG], fp32)
            nc.vector.tensor_tensor(m2[:], mean[:], mean[:], AOT.mult)
            vareps = small_p.tile([1, BAG], fp32)
            nc.vector.tensor_tensor(vareps[:], ex2[:], m2[:], AOT.subtract)
            nc.vector.tensor_scalar_add(vareps[:], vareps[:], EPS)
            rec = small_p.tile([1, BAG], fp32)
            nc.vector.reciprocal(rec[:], vareps[:])
            inv = small_p.tile([1, BAG], fp32)
            nc.scalar.sqrt(inv[:], rec[:])
            ab_row = small_p.tile([1, 2 * BAG], fp32)
            nc.vector.tensor_tensor(ab_row[0:1, 0:BAG], inv[:], grow[:], AOT.mult)
            mA = small_p.tile([1, BAG], fp32)
            nc.vector.tensor_tensor(mA[:], mean[:], ab_row[0:1, 0:BAG], AOT.mult)
            nc.vector.tensor_tensor(ab_row[0:1, BAG:2 * BAG], brow[:], mA[:],
                                    AOT.subtract)
            ab_ps = ab_p.tile([TILE, 2 * BAG], fp32)
            nc.tensor.matmul(ab_ps[:], ones_row[:], ab_row[:], start=True, stop=True)
            ab_sb = constp.tile([TILE, 2 * BAG], fp32)
            nc.vector.tensor_copy(ab_sb[:], ab_ps[:])

            # ---------------- phase 2: normalize + store ----------------
            a_col = ab_sb[:, 0:BAG]
            b_col = ab_sb[:, BAG:2 * BAG]
            w2 = 0
            while w2 < ng:
                nw = min(4, ng - w2)
                wid2 = nw * BAG
                arep = bass_mod.AP(tensor=a_col.tensor, offset=a_col.offset,
                                   ap=[a_col.ap[0], [0, nw], a_col.ap[1]])
                brep = bass_mod.AP(tensor=b_col.tensor, offset=b_col.offset,
                                   ap=[b_col.ap[0], [0, nw], b_col.ap[1]])
                ot = out_p.tile([TILE, 4 * BAG], fp32)
                src = agg_big[:, w2 * BAG:(w2 + nw) * BAG]
                nc.vector.tensor_tensor(
                    ot[:, 0:wid2].rearrange("p (a b) -> p a b", a=nw),
                    src.rearrange("p (a b) -> p a b", a=nw), arep, AOT.mult)
                nc.vector.tensor_tensor(
                    ot[:, 0:wid2].rearrange("p (a b) -> p a b", a=nw),
                    ot[:, 0:wid2].rearrange("p (a b) -> p a b", a=nw),
                    brep, AOT.add)
                # out rows for nw windows are contiguous: [w2*128, (w2+nw)*128)
                nc.sync.dma_start(
                    out[w2 * TILE:(w2 + nw) * TILE, :].rearrange(
                        "(a p) b -> p a b", p=TILE),
                    ot[:, 0:wid2].rearrange("p (a b) -> p a b", a=nw))
                w2 += nw

    nc.compile()
    return nc


# ----------------------------------------------------------------------------
# Entry point
# ----------------------------------------------------------------------------

def kernel(**inputs):
    global LAST_RESULTS
    from concourse.bass_utils import run_bass_kernel_spmd

    x = np.asarray(inputs["x"], dtype=np.float32)
    W = np.asarray(inputs["W"], dtype=np.float32)
    b = np.asarray(inputs["b"], dtype=np.float32)
    gamma = np.asarray(inputs["gamma"], dtype=np.float32)
    beta = np.asarray(inputs["beta"], dtype=np.float32)
    seg_ids = np.asarray(inputs["seg_ids"]).astype(np.int64)
    bags_len = np.asarray(inputs["bags_len"]).astype(np.int64)

    import os
    bf_mode = int(os.environ.get("KERNEL_BF16H", "4"))
    has_bias = bool(np.any(b != 0))
    if bf_mode >= 4 and has_bias:
        bf_mode = 3

    if bf_mode >= 4:
        plan, ng, in_maps, n_bags = _host_prep_t(
            x, W, gamma, beta, seg_ids, bags_len)
        key = (4, ng, n_bags)
        if key not in _NC_CACHE:
            _NC_CACHE[key] = _build_nc_t(ng, n_bags)
        nc = _NC_CACHE[key]
        res = run_bass_kernel_spmd(nc, in_maps, core_ids=list(range(N_CORES)))
        LAST_RESULTS = res
        out_full = np.zeros((n_bags, BAG), dtype=np.float32)
        for c in range(N_CORES):
            oc = np.asarray(res.results[c]["outT"]).astype(np.float32)
            for g, (p, e, fb, lbx) in enumerate(plan[c]["groups"]):
                ns = lbx - fb
                out_full[fb:lbx] = oc[:, g * SLOT_T: g * SLOT_T + ns].T
        return out_full

    plan, t0, ng, in_maps, n_bags = _host_prep(
        x, W, b, gamma, beta, seg_ids, bags_len, bf_mode)

    use_f32r = os.environ.get("KERNEL_F32R", "0") == "1"
    use_bf16h = 1 <= bf_mode <= 2
    use_bf16seg = bf_mode == 2 and not has_bias
    key = (ng, t0, n_bags, has_bias, use_f32r, bf_mode)
    if key not in _NC_CACHE:
        if bf_mode >= 3:
            _NC_CACHE[key] = _build_nc_pure(ng, t0, n_bags, has_bias)
        else:
            _NC_CACHE[key] = _build_nc(ng, t0, n_bags, has_bias,
                                       use_f32r=use_f32r, use_bf16h=use_bf16h,
                                       use_bf16seg=use_bf16seg)
    nc = _NC_CACHE[key]

    res = run_bass_kernel_spmd(nc, in_maps, core_ids=list(range(N_CORES)))
    LAST_RESULTS = res

    out_full = np.zeros((n_bags, BAG), dtype=np.float32)
    for c in range(N_CORES):
        oc = res.results[c]["out"]
        for g, (p, e, fb, lbx) in enumerate(plan[c]["groups"]):
            ns = lbx - fb
            out_full[fb:lbx] = oc[g * TILE: g * TILE + ns]
    return out_full
